# revision 17
# baseline (speedup 1.0000x reference)
"""Trainium2 Bass kernel for nn_MultiHeadedAttention_71425306132929.

Fused QKV projection + RoPE + causal/padding-masked SDPA + output projection.

Sharding: 8 cores = 2 batches x 4 head-groups (4 heads each).  Each core
computes, for its (batch, head-group):
    qkT = (Wq|Wk) @ query[b].T      (transposed layout: head-dim on partitions)
    RoPE on qT/kT via in-quadrant partition shuffle (head dims permuted
    host-side so RoPE partners are 16 partitions apart)
    scoresT[k,q] = kT.T-dot-qT per head (2 heads packed via PE row tiling)
    PT = exp(scoresT * 1/8)  (no max-subtraction needed: logits are O(1))
    causal masking: block-skip + trimmed exp/matmul APs + affine_select on
    the diagonal 128-blocks
    padding mask: folded into v (zeroed rows) + an extra all-mask column that
    makes the attention-denominator fall out of the same matmul
    ohT = (v|m).T @ PT accumulated over key blocks -> unnormalized out + denom
    normalize via reciprocal on a DMA-partition-packed view + broadcast loads
    yT_partial = WoutT.T @ ohT  (row-parallel out-projection, bf16 store)
Host sums the 4 partial yT per batch.

Scheduling: the attention stream is ACT(exp)-paced; emission order software-
pipelines scores one block ahead of PV and interleaves the next q-chunk's
projection matmuls + the previous chunk's out-projection as PE filler fed
from a generator queue.  At round boundaries only the two qk chains the next
round's first scores need are drained; the rest keeps filling.
"""

import os
import sys
from collections import deque

import numpy as np

sys.path.insert(0, "/opt/trn_rl_repo")

import concourse.bass as bass  # noqa: E402
import concourse.bacc as bacc  # noqa: E402
import concourse.tile as tile  # noqa: E402
from concourse import mybir  # noqa: E402

import ml_dtypes  # noqa: E402

BF16 = mybir.dt.bfloat16
F32 = mybir.dt.float32

B, S, DM, TD, H, HD = 2, 2048, 1024, 1024, 16, 64
NCORES = 8
NH = 4          # heads per core
NKB = S // 128  # 16 key blocks
NQC = S // 512  # 4 query chunks
KC = DM // 128  # 8 contraction chunks

# RoPE partner permutation: place original dim d so that partner(p) = p ^ 16
# (within a 32-partition quadrant, reachable by DVE stream_shuffle).
ROPE_PERM = []
for _p in range(64):
    q32, r32 = _p // 32, _p % 32
    ROPE_PERM.append(q32 * 16 + r32 if r32 < 16 else 32 + q32 * 16 + (r32 - 16))
ROPE_SGN = np.array([-1.0 if (p % 32) < 16 else 1.0 for p in range(64)], np.float32)
SHUF_MASK = [i ^ 16 for i in range(32)]

_CACHED = {}


class Gen:
    """Steppable wrapper over an emission generator."""

    def __init__(self, g):
        self.g = g
        self.done = False

    def step(self):
        if self.done:
            return False
        try:
            next(self.g)
            return True
        except StopIteration:
            self.done = True
            return False


def build_program(nvb=NKB):
    nc = bacc.Bacc(None, target_bir_lowering=False)
    qT_d = nc.declare_dram_parameter("qT", [DM, S], BF16, isOutput=False)
    wqk_d = nc.declare_dram_parameter("wqkT", [DM, 512], BF16, isOutput=False)
    wv_d = nc.declare_dram_parameter("wvT", [DM, 256], BF16, isOutput=False)
    cos_d = nc.declare_dram_parameter("cosT", [128, S], BF16, isOutput=False)
    sin_d = nc.declare_dram_parameter("sinT", [128, S], BF16, isOutput=False)
    mkv_d = nc.declare_dram_parameter("maskv", [128, NKB], F32, isOutput=False)
    wo_d = nc.declare_dram_parameter("woutT", [256, DM], BF16, isOutput=False)
    yT_d = nc.declare_dram_parameter("yT", [DM, S], BF16, isOutput=True)

    with tile.TileContext(nc) as tc:
        with (
            tc.tile_pool(name="const", bufs=1) as cpool,
            tc.tile_pool(name="work", bufs=1) as wpool,
            tc.tile_pool(name="rope", bufs=3) as rpool,
            tc.tile_pool(name="pt", bufs=6) as ptpool,
            tc.tile_pool(name="nrm", bufs=4) as npool,
            tc.tile_pool(name="yout", bufs=2) as ypool,
            tc.tile_pool(name="psA", bufs=2, space="PSUM") as psA,
            tc.tile_pool(name="psP", bufs=2, space="PSUM") as psP,
            tc.tile_pool(name="psO", bufs=2, space="PSUM") as psO,
        ):
            qT_sb = cpool.tile([128, KC, S], BF16, tag="qT")
            wqk_sb = cpool.tile([128, KC, 512], BF16, tag="wqk")
            wv_sb = cpool.tile([128, KC, 256], BF16, tag="wv")
            cos_sb = cpool.tile([128, S], BF16, tag="cos")
            sin_sb = cpool.tile([128, S], BF16, tag="sin")
            mkv_sb = cpool.tile([128, NKB], F32, tag="mkv")
            wo_sb = cpool.tile([128, 2, DM], BF16, tag="wo")

            qk_sb = wpool.tile([128, 4, S], BF16, tag="qk")
            vaug_sb = wpool.tile([128, NKB, 4, 128], BF16, tag="vaug")
            ohT_sb = wpool.tile([128, 2, S], BF16, tag="ohT")

            wqk_r = wqk_d.rearrange("(c p) s -> p c s", p=128)
            qT_r = qT_d.rearrange("(c p) s -> p c s", p=128)
            wv_r = wv_d.rearrange("(c p) s -> p c s", p=128)
            wo_r = wo_d.rearrange("(c p) s -> p c s", p=128)
            yT_r = yT_d.rearrange("(c p) s -> p c s", p=128)

            # Input DMA, merged into few descriptors, ordered so q-chunk
            # 0's projections can start ASAP.
            nc.sync.dma_start(mkv_sb[:], mkv_d[:])
            qsl0 = slice(0, 512)
            nc.sync.dma_start(wqk_sb[:], wqk_r[:])
            nc.sync.dma_start(qT_sb[:, :, qsl0], qT_r[:, :, qsl0])
            nc.sync.dma_start(wv_sb[:], wv_r[:])
            nc.sync.dma_start(cos_sb[:], cos_d[:])
            nc.sync.dma_start(sin_sb[:], sin_d[:])
            for qn in range(1, NQC):
                qsl = slice(qn * 512, qn * 512 + 512)
                nc.sync.dma_start(qT_sb[:, :, qsl], qT_r[:, :, qsl])
            nc.sync.dma_start(wo_sb[:], wo_r[:])

            # HAM warm-up: dependency-free matmuls keep the PE activity
            # monitor busy from t~=6us so the real prologue runs at 2.4GHz.
            warm_sb = cpool.tile([128, 128], BF16, tag="warm")
            nc.gpsimd.memset(warm_sb[:], 0.0)
            ones_sb = cpool.tile([128, 64], BF16, tag="ones")
            nc.gpsimd.memset(ones_sb[:], 1.0)
            warm_ps = psP.tile([128, 128], F32, tag="psP", name="warm_ps")
            for _w in range(64):
                nc.tensor.matmul(warm_ps[:], lhsT=warm_sb[:], rhs=warm_sb[:],
                                 start=True, stop=True, skip_group_check=True)

            nc.gpsimd.memset(vaug_sb[:], 0.0)
            # mask columns of v_aug: even slots col 64, odd slots col 32
            # (den must land on a legal engine start partition: 0/32/64/96)
            mkv_col = mkv_sb.rearrange("p (k o) -> p k o", o=1)
            nc.gpsimd.tensor_copy(vaug_sb[:, :, 0, 64:65], mkv_col)
            nc.gpsimd.tensor_copy(vaug_sb[:, :, 2, 64:65], mkv_col)
            nc.gpsimd.tensor_copy(vaug_sb[:, :, 1, 32:33], mkv_col)
            nc.gpsimd.tensor_copy(vaug_sb[:, :, 3, 32:33], mkv_col)

            def emit_qk(mt, qn):
                """project + rope one [128, 512] chunk of q or k (pair of heads).

                Generator: yields after each matmul so the caller can
                interleave; RoPE tail (DVE shuffle/mul + GPS mul/add) on close.
                """
                qsl = slice(qn * 512, qn * 512 + 512)
                ps = psP.tile([128, 512], F32, tag="psP")
                for kc in range(KC):
                    nc.tensor.matmul(
                        ps[:],
                        lhsT=wqk_sb[:, kc, mt * 128:(mt + 1) * 128],
                        rhs=qT_sb[:, kc, qsl],
                        start=(kc == 0),
                        stop=(kc == KC - 1),
                    )
                    if kc == 3:
                        yield
                qkp = rpool.tile([128, 512], BF16, tag="qkp")
                nc.vector.tensor_copy(qkp[:], ps[:])
                shuf = rpool.tile([128, 512], BF16, tag="shuf")
                nc.vector.stream_shuffle(shuf[:], qkp[:], mask=SHUF_MASK)
                t1 = rpool.tile([128, 512], BF16, tag="t1")
                nc.vector.tensor_mul(t1[:], qkp[:], cos_sb[:, qsl])
                t2 = rpool.tile([128, 512], BF16, tag="t2")
                nc.vector.tensor_mul(t2[:], shuf[:], sin_sb[:, qsl])
                nc.vector.tensor_add(qk_sb[:, mt, qsl], t1[:], t2[:])
                yield

            def emit_v(st):
                """project + mask one [128 keys, 4*64] v block into v_aug."""
                ps = psP.tile([128, 512], F32, tag="psP")
                psv = ps[:, 0:256]
                for kc in range(KC):
                    nc.tensor.matmul(
                        psv,
                        lhsT=qT_sb[:, kc, st * 128:(st + 1) * 128],
                        rhs=wv_sb[:, kc, :],
                        start=(kc == 0),
                        stop=(kc == KC - 1),
                    )
                    if kc == 3:
                        yield
                psv_h = psv.rearrange("p (h d) -> p h d", h=4)
                msk = mkv_sb[:, st:st + 1]
                # even local heads (slots 0,2) -> cols 0:64 ; odd -> cols 64:128
                nc.vector.tensor_scalar_mul(
                    vaug_sb[:, st, 0:4:2, 0:64], psv_h[:, 0:4:2, :], msk)
                nc.vector.tensor_scalar_mul(
                    vaug_sb[:, st, 1:4:2, 64:128], psv_h[:, 1:4:2, :], msk)
                yield

            def emit_outproj(qn):
                """row-parallel out-projection of one q chunk (bf16 store)."""
                qsl = slice(qn * 512, qn * 512 + 512)
                y = ypool.tile([128, 8, 512], BF16, tag="y")
                for mt in range(8):
                    ps = psP.tile([128, 512], F32, tag="psP")
                    for kc2 in range(2):
                        nc.tensor.matmul(
                            ps[:],
                            lhsT=wo_sb[:, kc2, mt * 128:(mt + 1) * 128],
                            rhs=ohT_sb[:, kc2, qsl],
                            start=(kc2 == 0),
                            stop=(kc2 == 1),
                        )
                    nc.any.tensor_copy(y[:, mt, :], ps[:])
                    if mt % 2 == 1:
                        nc.sync.dma_start(yT_r[:, mt - 1:mt + 1, qsl],
                                          y[:, mt - 1:mt + 1, :])
                    yield

            # ---- filler queue: projection / out-projection emission units
            # interleaved into the ACT-paced attention stream ----
            filler = deque()

            def pump(n):
                done = 0
                while done < n and filler:
                    if filler[0].step():
                        done += 1
                    else:
                        filler.popleft()

            def drain(*gens):
                for g in gens:
                    while g.step():
                        pass

            def flush():
                while filler:
                    if not filler[0].step():
                        filler.popleft()

            def queue_prep(qc):
                """queue next q-chunk's projections; returns handles:
                (pair0 qk chains, v chains, pair1 qk chains).  k/v chains
                past the padding-valid key range are never needed."""
                p0 = [Gen(emit_qk(0, qc))]
                p1 = [Gen(emit_qk(1, qc))]
                if qc * 512 < nvb * 128:
                    p0.insert(0, Gen(emit_qk(2, qc)))
                    p1.insert(0, Gen(emit_qk(3, qc)))
                vs = [Gen(emit_v(st))
                      for st in range(4 * qc, 4 * qc + 4) if st < nvb]
                for g in p0 + vs + p1:
                    filler.append(g)
                return p0, vs, p1

            def emit_attn(pair, qc, v_gens=None):
                """ACT-paced attention for one head-pair and q chunk.

                Software pipeline: scores one block ahead of PV; exp and
                matmul APs trimmed to the causally-valid q columns.
                v_gens: this chunk's diagonal v-projection chains — drained
                just before the first diagonal block's PV can need them.
                """
                nkb = min(4 * qc + 4, nvb)
                qmt, kmt = pair, 2 + pair
                oT = [psO.tile([128, 512], F32, tag="psO", name=f"oT{_h}")
                      for _h in range(2)]
                pts = [None] * nkb   # (pt tile, co) per block
                for kb in range(nkb):
                    if kb == max(4 * qc - 1, 0) and v_gens:
                        drain(*v_gens)
                    ksl = slice(kb * 128, kb * 128 + 128)
                    joff = kb - 4 * qc
                    co = max(joff, 0) * 128  # first causally-valid q col
                    st_ps = psA.tile([128, 1024], F32, tag="psA", name="stps")
                    for h in range(2):
                        pr = slice(64 * h, 64 * h + 64)
                        nc.tensor.matmul(
                            st_ps[:, h * 512 + co:(h + 1) * 512],
                            lhsT=qk_sb[pr, kmt, ksl],
                            rhs=qk_sb[pr, qmt, qc * 512 + co:qc * 512 + 512],
                            start=True,
                            stop=True,
                            skip_group_check=True,
                        )
                    pt = ptpool.tile([128, 1024], BF16, tag="pt", name="pt")
                    pts[kb] = (pt, co)
                    if co == 0:
                        nc.scalar.activation(
                            pt[:], st_ps[:],
                            mybir.ActivationFunctionType.Exp,
                            scale=0.125,
                        )
                    else:
                        # strided 2-range AP: only the valid q cols per head
                        pt_v = pt.rearrange("p (h q) -> p h q", h=2)
                        st_v = st_ps.rearrange("p (h q) -> p h q", h=2)
                        nc.scalar.activation(
                            pt_v[:, :, co:512], st_v[:, :, co:512],
                            mybir.ActivationFunctionType.Exp,
                            scale=0.125,
                        )
                    if joff >= 0:
                        for h in range(2):
                            nc.gpsimd.affine_select(
                                pt[:, h * 512 + co:h * 512 + co + 128],
                                pt[:, h * 512 + co:h * 512 + co + 128],
                                pattern=[[1, 128]],
                                compare_op=mybir.AluOpType.is_ge,
                                fill=0.0,
                                base=0,
                                channel_multiplier=-1,
                            )
                    if kb >= 1:
                        emit_pv(pair, kb - 1, pts[kb - 1], oT, nkb)
                    pump(1)
                emit_pv(pair, nkb - 1, pts[nkb - 1], oT, nkb)
                emit_norm(pair, qc, oT)

            def emit_pv(pair, kb, pt_co, oT, nkb):
                pt, co = pt_co
                for h in range(2):
                    nc.tensor.matmul(
                        oT[h][:, co:512],
                        lhsT=vaug_sb[:, kb, 2 * pair + h, :],
                        rhs=pt[:, h * 512 + co:(h + 1) * 512],
                        start=(kb == 0),
                        stop=(kb == nkb - 1),
                        skip_group_check=True,
                    )

            def emit_norm(pair, qc, oT):
                """normalize fully on-chip: SBUF->SBUF DMA packs the two
                [1,512] den rows into [128,8] lanes for a cheap reciprocal,
                unpacks, then a ones-matmul broadcasts across partitions."""
                qsl = slice(qc * 512, qc * 512 + 512)
                osb = []
                for h in range(2):
                    o = npool.tile([128, 512], F32, tag="osb", name=f"osb{h}")
                    nc.vector.tensor_copy(o[:], oT[h][:])  # frees the psum bank
                    osb.append(o)
                rcp = npool.tile([128, 8], F32, tag="rcp")
                nc.sync.dma_start(rcp[0:64, :], osb[0][64:65, :])
                nc.sync.dma_start(rcp[64:128, :], osb[1][32:33, :])
                rcp2 = npool.tile([128, 8], BF16, tag="rcp2")
                with nc.allow_low_precision(reason="den recip in bf16: 0.4% on a 2e-2 budget"):
                    nc.vector.reciprocal(rcp2[:], rcp[:])
                rrow = npool.tile([128, 512], BF16, tag="rrow")
                nc.sync.dma_start(rrow[64:65, :], rcp2[0:64, :])
                nc.sync.dma_start(rrow[32:33, :], rcp2[64:128, :])
                bc = psP.tile([128, 512], F32, tag="psP", name="bc")
                nc.tensor.matmul(
                    bc[0:64, :], lhsT=ones_sb[64:65, :], rhs=rrow[64:65, :],
                    start=True, stop=True, tile_position=(64, 0),
                    skip_group_check=True)
                nc.tensor.matmul(
                    bc[64:128, :], lhsT=ones_sb[32:33, :], rhs=rrow[32:33, :],
                    start=True, stop=True, tile_position=(32, 64),
                    skip_group_check=True)
                nc.vector.tensor_mul(
                    ohT_sb[0:64, pair, qsl], osb[0][0:64, :], bc[0:64, :])
                nc.vector.tensor_mul(
                    ohT_sb[64:128, pair, qsl], osb[1][64:128, :], bc[64:128, :])

            # ---- main schedule ----
            # prologue: only the two qk chains attn(0,0) needs, drained;
            # the rest of prep(0) goes through the filler queue.
            g20 = Gen(emit_qk(2, 0))
            g00 = Gen(emit_qk(0, 0))
            drain(g20, g00)
            vs = [Gen(emit_v(st)) for st in range(4)]
            p1 = [Gen(emit_qk(3, 0)), Gen(emit_qk(1, 0))]
            for g in vs + p1:
                filler.append(g)

            for qc in range(NQC):
                nxt = queue_prep(qc + 1) if qc + 1 < NQC else None
                emit_attn(0, qc, vs)
                drain(*p1)
                emit_attn(1, qc, vs)
                filler.append(Gen(emit_outproj(qc)))
                if nxt is not None:
                    p0, vs, p1 = nxt
                    drain(*p0)  # the two chains attn(0, qc+1) starts on
            flush()

    nc.compile()
    return nc


def make_in_maps(query, W_in, W_out, sin_q, cos_q, attn_mask):
    bf = ml_dtypes.bfloat16
    cosT = np.asarray(cos_q, np.float32)[0, 0].T  # [64, S]
    sinT = np.asarray(sin_q, np.float32)[0, 0].T
    cosT_p = cosT[ROPE_PERM]
    sinT_p = sinT[ROPE_PERM] * ROPE_SGN[:, None]
    cos2 = np.concatenate([cosT_p, cosT_p], 0).astype(bf)    # [128, S]
    sin2 = np.concatenate([sinT_p, sinT_p], 0).astype(bf)
    W_in = np.asarray(W_in, np.float32)
    W_out = np.asarray(W_out, np.float32)
    query = np.asarray(query, np.float32)
    attn_mask = np.asarray(attn_mask)

    in_maps = []
    for c in range(NCORES):
        b, g = c // 4, c % 4
        heads = range(4 * g, 4 * g + 4)
        qrows = np.concatenate([W_in[h * 64:(h + 1) * 64][ROPE_PERM] for h in heads])
        krows = np.concatenate([W_in[TD + h * 64:TD + (h + 1) * 64][ROPE_PERM] for h in heads])
        vrows = np.concatenate([W_in[2 * TD + h * 64:2 * TD + (h + 1) * 64] for h in heads])
        tcols = np.concatenate([np.arange(h * 64, (h + 1) * 64) for h in heads])
        in_maps.append({
            "qT": np.ascontiguousarray(query[b].T).astype(bf),
            "wqkT": np.ascontiguousarray(np.concatenate([qrows, krows], 0).T).astype(bf),
            "wvT": np.ascontiguousarray(vrows.T).astype(bf),
            "cosT": cos2,
            "sinT": sin2,
            "maskv": np.ascontiguousarray(
                attn_mask[b].astype(np.float32).reshape(NKB, 128).T),
            "woutT": np.ascontiguousarray(W_out[:, tcols].T).astype(bf),
        })
    return in_maps


def _ensure_ntff_hook():
    """The image's antenv lacks axon_hooks; supply it so trace=True works."""
    try:
        from antenv.axon_hooks import get_axon_ntff_profile_hook  # noqa: F401
        return
    except ImportError:
        pass
    import types

    if "/root/.axon_site" not in sys.path:
        sys.path.insert(0, "/root/.axon_site")
    from trn_agent_boot.trn_boot import _ntff_profile_via_ctypes

    hook = _ntff_profile_via_ctypes("/opt/axon/libaxon_pjrt.so")
    mod = types.ModuleType("antenv.axon_hooks")
    mod._hook = hook
    mod.get_axon_ntff_profile_hook = lambda: mod._hook
    mod.set_axon_ntff_profile_hook = lambda h: setattr(mod, "_hook", h)
    sys.modules["antenv.axon_hooks"] = mod
    import antenv

    antenv.axon_hooks = mod


def kernel(query, W_in, W_out, sin_q, cos_q, attn_mask):
    mask = np.asarray(attn_mask)
    nvb = 1
    for b in range(B):
        idx = np.nonzero(mask[b])[0]
        last = int(idx[-1]) if idx.size else 0
        nvb = max(nvb, last // 128 + 1)
    key = ("nc", nvb)
    if key not in _CACHED:
        _CACHED[key] = build_program(nvb)
    nc = _CACHED[key]
    in_maps = make_in_maps(query, W_in, W_out, sin_q, cos_q, attn_mask)

    from concourse.bass_utils import run_bass_kernel_spmd

    trace = bool(os.environ.get("KERNEL_PROFILE"))
    if trace:
        try:
            _ensure_ntff_hook()
        except Exception as e:  # profiling is best-effort
            print(f"ntff hook unavailable: {e}")
            trace = False
    try:
        res = run_bass_kernel_spmd(nc, in_maps, list(range(NCORES)), trace=trace)
    except Exception:
        if not trace:
            raise
        res = run_bass_kernel_spmd(nc, in_maps, list(range(NCORES)), trace=False)
    _CACHED["last_result"] = res

    y = np.zeros((B, S, DM), np.float32)
    for c in range(NCORES):
        y[c // 4] += res.results[c]["yT"].astype(np.float32).T
    return y


# revision 18
# speedup vs baseline: 1.2470x; 1.2470x over previous
"""Trainium2 Bass kernel for nn_MultiHeadedAttention_71425306132929.

Fused QKV projection + RoPE + causal/padding-masked SDPA + output projection.

Sharding: 8 cores = 2 batches x 4 head-groups (4 heads each).  Each core
computes, for its (batch, head-group):
    qkT = (Wq|Wk) @ query[b].T      (transposed layout: head-dim on partitions)
    RoPE on qT/kT via in-quadrant partition shuffle (head dims permuted
    host-side so RoPE partners are 16 partitions apart)
    scoresT[k,q] = kT.T-dot-qT per head (2 heads packed via PE row tiling)
    PT = exp(scoresT * 1/8)  (no max-subtraction needed: logits are O(1))
    causal masking: block-skip + trimmed exp/matmul APs + affine_select on
    the diagonal 128-blocks
    padding mask: folded into v (zeroed rows) + an extra all-mask column that
    makes the attention-denominator fall out of the same matmul
    ohT = (v|m).T @ PT accumulated over key blocks -> unnormalized out + denom
    normalize via reciprocal on a DMA-partition-packed view + broadcast loads
    yT_partial = WoutT.T @ ohT  (row-parallel out-projection, bf16 store)
Host sums the 4 partial yT per batch.

Scheduling: the attention stream is ACT(exp)-paced; emission order software-
pipelines scores one block ahead of PV and interleaves the next q-chunk's
projection matmuls + the previous chunk's out-projection as PE filler fed
from a generator queue.  At round boundaries only the two qk chains the next
round's first scores need are drained; the rest keeps filling.
"""

import os
import sys
from collections import deque

import numpy as np

sys.path.insert(0, "/opt/trn_rl_repo")

import concourse.bass as bass  # noqa: E402
import concourse.bacc as bacc  # noqa: E402
import concourse.tile as tile  # noqa: E402
from concourse import mybir  # noqa: E402

import ml_dtypes  # noqa: E402

BF16 = mybir.dt.bfloat16
F32 = mybir.dt.float32

B, S, DM, TD, H, HD = 2, 2048, 1024, 1024, 16, 64
NCORES = 8
NH = 4          # heads per core
NKB = S // 128  # 16 key blocks
NQC = S // 512  # 4 query chunks
KC = DM // 128  # 8 contraction chunks

# RoPE partner permutation: place original dim d so that partner(p) = p ^ 16
# (within a 32-partition quadrant, reachable by DVE stream_shuffle).
ROPE_PERM = []
for _p in range(64):
    q32, r32 = _p // 32, _p % 32
    ROPE_PERM.append(q32 * 16 + r32 if r32 < 16 else 32 + q32 * 16 + (r32 - 16))
ROPE_SGN = np.array([-1.0 if (p % 32) < 16 else 1.0 for p in range(64)], np.float32)
SHUF_MASK = [i ^ 16 for i in range(32)]

_CACHED = {}


class Gen:
    """Steppable wrapper over an emission generator."""

    def __init__(self, g):
        self.g = g
        self.done = False

    def step(self):
        if self.done:
            return False
        try:
            next(self.g)
            return True
        except StopIteration:
            self.done = True
            return False


def build_program(nvb=NKB):
    nc = bacc.Bacc(None, target_bir_lowering=False)
    qT_d = nc.declare_dram_parameter("qT", [DM, S], BF16, isOutput=False)
    wqk_d = nc.declare_dram_parameter("wqkT", [DM, 512], BF16, isOutput=False)
    wv_d = nc.declare_dram_parameter("wvT", [DM, 256], BF16, isOutput=False)
    cos_d = nc.declare_dram_parameter("cosT", [128, S], BF16, isOutput=False)
    sin_d = nc.declare_dram_parameter("sinT", [128, S], BF16, isOutput=False)
    mkv_d = nc.declare_dram_parameter("maskv", [128, NKB], F32, isOutput=False)
    wo_d = nc.declare_dram_parameter("woutT", [256, DM], BF16, isOutput=False)
    yT_d = nc.declare_dram_parameter("yT", [DM, S], BF16, isOutput=True)
    dscr2 = nc.dram_tensor("rcp_scratch", [16, 512], F32)

    with tile.TileContext(nc) as tc:
        with (
            tc.tile_pool(name="const", bufs=1) as cpool,
            tc.tile_pool(name="work", bufs=1) as wpool,
            tc.tile_pool(name="rope", bufs=3) as rpool,
            tc.tile_pool(name="pt", bufs=6) as ptpool,
            tc.tile_pool(name="nrm", bufs=4) as npool,
            tc.tile_pool(name="yout", bufs=2) as ypool,
            tc.tile_pool(name="psA", bufs=2, space="PSUM") as psA,
            tc.tile_pool(name="psP", bufs=2, space="PSUM") as psP,
            tc.tile_pool(name="psO", bufs=2, space="PSUM") as psO,
        ):
            qT_sb = cpool.tile([128, KC, S], BF16, tag="qT")
            wqk_sb = cpool.tile([128, KC, 512], BF16, tag="wqk")
            wv_sb = cpool.tile([128, KC, 256], BF16, tag="wv")
            cos_sb = cpool.tile([128, S], BF16, tag="cos")
            sin_sb = cpool.tile([128, S], BF16, tag="sin")
            mkv_sb = cpool.tile([128, NKB], F32, tag="mkv")
            wo_sb = cpool.tile([128, 2, DM], BF16, tag="wo")

            qk_sb = wpool.tile([128, 4, S], BF16, tag="qk")
            vaug_sb = wpool.tile([128, NKB, 4, 128], BF16, tag="vaug")
            ohT_sb = wpool.tile([128, 2, S], BF16, tag="ohT")

            wqk_r = wqk_d.rearrange("(c p) s -> p c s", p=128)
            qT_r = qT_d.rearrange("(c p) s -> p c s", p=128)
            wv_r = wv_d.rearrange("(c p) s -> p c s", p=128)
            wo_r = wo_d.rearrange("(c p) s -> p c s", p=128)
            yT_r = yT_d.rearrange("(c p) s -> p c s", p=128)

            # Input DMA, merged into few descriptors, ordered so q-chunk
            # 0's projections can start ASAP.
            nc.sync.dma_start(mkv_sb[:], mkv_d[:])
            qsl0 = slice(0, 512)
            nc.sync.dma_start(wqk_sb[:], wqk_r[:])
            nc.sync.dma_start(qT_sb[:, :, qsl0], qT_r[:, :, qsl0])
            nc.sync.dma_start(wv_sb[:], wv_r[:])
            nc.sync.dma_start(cos_sb[:], cos_d[:])
            nc.sync.dma_start(sin_sb[:], sin_d[:])
            for qn in range(1, NQC):
                qsl = slice(qn * 512, qn * 512 + 512)
                nc.sync.dma_start(qT_sb[:, :, qsl], qT_r[:, :, qsl])
            nc.sync.dma_start(wo_sb[:], wo_r[:])

            # HAM warm-up: dependency-free matmuls keep the PE activity
            # monitor busy from t~=6us so the real prologue runs at 2.4GHz.
            warm_sb = cpool.tile([128, 128], BF16, tag="warm")
            nc.gpsimd.memset(warm_sb[:], 0.0)
            warm_ps = psP.tile([128, 128], F32, tag="psP", name="warm_ps")
            for _w in range(64):
                nc.tensor.matmul(warm_ps[:], lhsT=warm_sb[:], rhs=warm_sb[:],
                                 start=True, stop=True, skip_group_check=True)

            nc.gpsimd.memset(vaug_sb[:], 0.0)
            # mask columns of v_aug: even slots col 64, odd slots col 32
            # (den must land on a legal engine start partition: 0/32/64/96)
            mkv_col = mkv_sb.rearrange("p (k o) -> p k o", o=1)
            nc.gpsimd.tensor_copy(vaug_sb[:, :, 0, 64:65], mkv_col)
            nc.gpsimd.tensor_copy(vaug_sb[:, :, 2, 64:65], mkv_col)
            nc.gpsimd.tensor_copy(vaug_sb[:, :, 1, 32:33], mkv_col)
            nc.gpsimd.tensor_copy(vaug_sb[:, :, 3, 32:33], mkv_col)

            def emit_qk(mt, qn):
                """project + rope one [128, 512] chunk of q or k (pair of heads).

                Generator: yields after each matmul so the caller can
                interleave; RoPE tail (DVE shuffle/mul + GPS mul/add) on close.
                """
                qsl = slice(qn * 512, qn * 512 + 512)
                ps = psP.tile([128, 512], F32, tag="psP")
                for kc in range(KC):
                    nc.tensor.matmul(
                        ps[:],
                        lhsT=wqk_sb[:, kc, mt * 128:(mt + 1) * 128],
                        rhs=qT_sb[:, kc, qsl],
                        start=(kc == 0),
                        stop=(kc == KC - 1),
                    )
                    if kc == 3:
                        yield
                qkp = rpool.tile([128, 512], BF16, tag="qkp")
                nc.vector.tensor_copy(qkp[:], ps[:])
                shuf = rpool.tile([128, 512], BF16, tag="shuf")
                nc.vector.stream_shuffle(shuf[:], qkp[:], mask=SHUF_MASK)
                t1 = rpool.tile([128, 512], BF16, tag="t1")
                nc.vector.tensor_mul(t1[:], qkp[:], cos_sb[:, qsl])
                t2 = rpool.tile([128, 512], BF16, tag="t2")
                nc.vector.tensor_mul(t2[:], shuf[:], sin_sb[:, qsl])
                nc.vector.tensor_add(qk_sb[:, mt, qsl], t1[:], t2[:])
                yield

            def emit_v(st):
                """project + mask one [128 keys, 4*64] v block into v_aug."""
                ps = psP.tile([128, 512], F32, tag="psP")
                psv = ps[:, 0:256]
                for kc in range(KC):
                    nc.tensor.matmul(
                        psv,
                        lhsT=qT_sb[:, kc, st * 128:(st + 1) * 128],
                        rhs=wv_sb[:, kc, :],
                        start=(kc == 0),
                        stop=(kc == KC - 1),
                    )
                    if kc == 3:
                        yield
                psv_h = psv.rearrange("p (h d) -> p h d", h=4)
                msk = mkv_sb[:, st:st + 1]
                # even local heads (slots 0,2) -> cols 0:64 ; odd -> cols 64:128
                nc.vector.tensor_scalar_mul(
                    vaug_sb[:, st, 0:4:2, 0:64], psv_h[:, 0:4:2, :], msk)
                nc.vector.tensor_scalar_mul(
                    vaug_sb[:, st, 1:4:2, 64:128], psv_h[:, 1:4:2, :], msk)
                yield

            def emit_outproj(qn):
                """row-parallel out-projection of one q chunk (bf16 store)."""
                qsl = slice(qn * 512, qn * 512 + 512)
                y = ypool.tile([128, 8, 512], BF16, tag="y")
                for mt in range(8):
                    ps = psP.tile([128, 512], F32, tag="psP")
                    for kc2 in range(2):
                        nc.tensor.matmul(
                            ps[:],
                            lhsT=wo_sb[:, kc2, mt * 128:(mt + 1) * 128],
                            rhs=ohT_sb[:, kc2, qsl],
                            start=(kc2 == 0),
                            stop=(kc2 == 1),
                        )
                    nc.any.tensor_copy(y[:, mt, :], ps[:])
                    if mt % 2 == 1:
                        nc.sync.dma_start(yT_r[:, mt - 1:mt + 1, qsl],
                                          y[:, mt - 1:mt + 1, :])
                    yield

            # ---- filler queue: projection / out-projection emission units
            # interleaved into the ACT-paced attention stream ----
            filler = deque()

            def pump(n):
                done = 0
                while done < n and filler:
                    if filler[0].step():
                        done += 1
                    else:
                        filler.popleft()

            def drain(*gens):
                for g in gens:
                    while g.step():
                        pass

            def flush():
                while filler:
                    if not filler[0].step():
                        filler.popleft()

            def queue_prep(qc):
                """queue next q-chunk's projections; returns handles:
                (pair0 qk chains, v chains, pair1 qk chains).  k/v chains
                past the padding-valid key range are never needed."""
                p0 = [Gen(emit_qk(0, qc))]
                p1 = [Gen(emit_qk(1, qc))]
                if qc * 512 < nvb * 128:
                    p0.insert(0, Gen(emit_qk(2, qc)))
                    p1.insert(0, Gen(emit_qk(3, qc)))
                vs = [Gen(emit_v(st))
                      for st in range(4 * qc, 4 * qc + 4) if st < nvb]
                for g in p0 + vs + p1:
                    filler.append(g)
                return p0, vs, p1

            def emit_attn(pair, qc, v_gens=None):
                """ACT-paced attention for one head-pair and q chunk.

                Software pipeline: scores one block ahead of PV; exp and
                matmul APs trimmed to the causally-valid q columns.
                v_gens: this chunk's diagonal v-projection chains — drained
                just before the first diagonal block's PV can need them.
                """
                nkb = min(4 * qc + 4, nvb)
                qmt, kmt = pair, 2 + pair
                oT = [psO.tile([128, 512], F32, tag="psO", name=f"oT{_h}")
                      for _h in range(2)]
                pts = [None] * nkb   # (pt tile, co) per block
                for kb in range(nkb):
                    if kb == max(4 * qc - 1, 0) and v_gens:
                        drain(*v_gens)
                    ksl = slice(kb * 128, kb * 128 + 128)
                    joff = kb - 4 * qc
                    co = max(joff, 0) * 128  # first causally-valid q col
                    st_ps = psA.tile([128, 1024], F32, tag="psA", name="stps")
                    for h in range(2):
                        pr = slice(64 * h, 64 * h + 64)
                        nc.tensor.matmul(
                            st_ps[:, h * 512 + co:(h + 1) * 512],
                            lhsT=qk_sb[pr, kmt, ksl],
                            rhs=qk_sb[pr, qmt, qc * 512 + co:qc * 512 + 512],
                            start=True,
                            stop=True,
                            skip_group_check=True,
                        )
                    pt = ptpool.tile([128, 1024], BF16, tag="pt", name="pt")
                    pts[kb] = (pt, co)
                    if co == 0:
                        nc.scalar.activation(
                            pt[:], st_ps[:],
                            mybir.ActivationFunctionType.Exp,
                            scale=0.125,
                        )
                    else:
                        # strided 2-range AP: only the valid q cols per head
                        pt_v = pt.rearrange("p (h q) -> p h q", h=2)
                        st_v = st_ps.rearrange("p (h q) -> p h q", h=2)
                        nc.scalar.activation(
                            pt_v[:, :, co:512], st_v[:, :, co:512],
                            mybir.ActivationFunctionType.Exp,
                            scale=0.125,
                        )
                    if joff >= 0:
                        for h in range(2):
                            nc.gpsimd.affine_select(
                                pt[:, h * 512 + co:h * 512 + co + 128],
                                pt[:, h * 512 + co:h * 512 + co + 128],
                                pattern=[[1, 128]],
                                compare_op=mybir.AluOpType.is_ge,
                                fill=0.0,
                                base=0,
                                channel_multiplier=-1,
                            )
                    if kb >= 1:
                        emit_pv(pair, kb - 1, pts[kb - 1], oT, nkb)
                    pump(1)
                emit_pv(pair, nkb - 1, pts[nkb - 1], oT, nkb)
                emit_norm(pair, qc, oT)

            def emit_pv(pair, kb, pt_co, oT, nkb):
                pt, co = pt_co
                for h in range(2):
                    nc.tensor.matmul(
                        oT[h][:, co:512],
                        lhsT=vaug_sb[:, kb, 2 * pair + h, :],
                        rhs=pt[:, h * 512 + co:(h + 1) * 512],
                        start=(kb == 0),
                        stop=(kb == nkb - 1),
                        skip_group_check=True,
                    )

            def emit_norm(pair, qc, oT):
                """normalize: SBUF->SBUF DMA packs the two [1,512] den rows
                into [128,8] lanes for a cheap reciprocal, then a DRAM
                bounce provides the partition-broadcast.  No PE instruction
                in this chain: the in-order PE stream must never block on
                DMA latency."""
                qsl = slice(qc * 512, qc * 512 + 512)
                base = (pair * 4 + qc) * 2
                osb = []
                for h in range(2):
                    o = npool.tile([128, 512], F32, tag="osb", name=f"osb{h}")
                    nc.vector.tensor_copy(o[:], oT[h][:])  # frees the psum bank
                    osb.append(o)
                rcp = npool.tile([128, 8], F32, tag="rcp")
                nc.sync.dma_start(rcp[0:64, :], osb[0][64:65, :])
                nc.sync.dma_start(rcp[64:128, :], osb[1][32:33, :])
                rcp2 = npool.tile([128, 8], F32, tag="rcp2")
                nc.vector.reciprocal(rcp2[:], rcp[:])
                nc.sync.dma_start(
                    dscr2[base:base + 2, :].rearrange("a (p f) -> (a p) f", f=8), rcp2[:])
                bc = npool.tile([128, 512], F32, tag="bc")
                nc.gpsimd.dma_start(bc[0:64, :],
                                    dscr2[base:base + 1, :].to_broadcast((64, 512)))
                nc.gpsimd.dma_start(bc[64:128, :],
                                    dscr2[base + 1:base + 2, :].to_broadcast((64, 512)))
                nc.vector.tensor_mul(
                    ohT_sb[0:64, pair, qsl], osb[0][0:64, :], bc[0:64, :])
                nc.vector.tensor_mul(
                    ohT_sb[64:128, pair, qsl], osb[1][64:128, :], bc[64:128, :])

            # ---- main schedule ----
            # prologue: only the two qk chains attn(0,0) needs, drained;
            # the rest of prep(0) goes through the filler queue.
            g20 = Gen(emit_qk(2, 0))
            g00 = Gen(emit_qk(0, 0))
            drain(g20, g00)
            vs = [Gen(emit_v(st)) for st in range(4)]
            p1 = [Gen(emit_qk(3, 0)), Gen(emit_qk(1, 0))]
            for g in vs + p1:
                filler.append(g)

            for qc in range(NQC):
                nxt = queue_prep(qc + 1) if qc + 1 < NQC else None
                emit_attn(0, qc, vs)
                drain(*p1)
                emit_attn(1, qc, vs)
                filler.append(Gen(emit_outproj(qc)))
                if nxt is not None:
                    p0, vs, p1 = nxt
                    drain(*p0)  # the two chains attn(0, qc+1) starts on
            flush()

    nc.compile()
    return nc


def make_in_maps(query, W_in, W_out, sin_q, cos_q, attn_mask):
    bf = ml_dtypes.bfloat16
    cosT = np.asarray(cos_q, np.float32)[0, 0].T  # [64, S]
    sinT = np.asarray(sin_q, np.float32)[0, 0].T
    cosT_p = cosT[ROPE_PERM]
    sinT_p = sinT[ROPE_PERM] * ROPE_SGN[:, None]
    cos2 = np.concatenate([cosT_p, cosT_p], 0).astype(bf)    # [128, S]
    sin2 = np.concatenate([sinT_p, sinT_p], 0).astype(bf)
    W_in = np.asarray(W_in, np.float32)
    W_out = np.asarray(W_out, np.float32)
    query = np.asarray(query, np.float32)
    attn_mask = np.asarray(attn_mask)

    in_maps = []
    for c in range(NCORES):
        b, g = c // 4, c % 4
        heads = range(4 * g, 4 * g + 4)
        qrows = np.concatenate([W_in[h * 64:(h + 1) * 64][ROPE_PERM] for h in heads])
        krows = np.concatenate([W_in[TD + h * 64:TD + (h + 1) * 64][ROPE_PERM] for h in heads])
        vrows = np.concatenate([W_in[2 * TD + h * 64:2 * TD + (h + 1) * 64] for h in heads])
        tcols = np.concatenate([np.arange(h * 64, (h + 1) * 64) for h in heads])
        in_maps.append({
            "qT": np.ascontiguousarray(query[b].T).astype(bf),
            "wqkT": np.ascontiguousarray(np.concatenate([qrows, krows], 0).T).astype(bf),
            "wvT": np.ascontiguousarray(vrows.T).astype(bf),
            "cosT": cos2,
            "sinT": sin2,
            "maskv": np.ascontiguousarray(
                attn_mask[b].astype(np.float32).reshape(NKB, 128).T),
            "woutT": np.ascontiguousarray(W_out[:, tcols].T).astype(bf),
        })
    return in_maps


def _ensure_ntff_hook():
    """The image's antenv lacks axon_hooks; supply it so trace=True works."""
    try:
        from antenv.axon_hooks import get_axon_ntff_profile_hook  # noqa: F401
        return
    except ImportError:
        pass
    import types

    if "/root/.axon_site" not in sys.path:
        sys.path.insert(0, "/root/.axon_site")
    from trn_agent_boot.trn_boot import _ntff_profile_via_ctypes

    hook = _ntff_profile_via_ctypes("/opt/axon/libaxon_pjrt.so")
    mod = types.ModuleType("antenv.axon_hooks")
    mod._hook = hook
    mod.get_axon_ntff_profile_hook = lambda: mod._hook
    mod.set_axon_ntff_profile_hook = lambda h: setattr(mod, "_hook", h)
    sys.modules["antenv.axon_hooks"] = mod
    import antenv

    antenv.axon_hooks = mod


def kernel(query, W_in, W_out, sin_q, cos_q, attn_mask):
    mask = np.asarray(attn_mask)
    nvb = 1
    for b in range(B):
        idx = np.nonzero(mask[b])[0]
        last = int(idx[-1]) if idx.size else 0
        nvb = max(nvb, last // 128 + 1)
    key = ("nc", nvb)
    if key not in _CACHED:
        _CACHED[key] = build_program(nvb)
    nc = _CACHED[key]
    in_maps = make_in_maps(query, W_in, W_out, sin_q, cos_q, attn_mask)

    from concourse.bass_utils import run_bass_kernel_spmd

    trace = bool(os.environ.get("KERNEL_PROFILE"))
    if trace:
        try:
            _ensure_ntff_hook()
        except Exception as e:  # profiling is best-effort
            print(f"ntff hook unavailable: {e}")
            trace = False
    try:
        res = run_bass_kernel_spmd(nc, in_maps, list(range(NCORES)), trace=trace)
    except Exception:
        if not trace:
            raise
        res = run_bass_kernel_spmd(nc, in_maps, list(range(NCORES)), trace=False)
    _CACHED["last_result"] = res

    y = np.zeros((B, S, DM), np.float32)
    for c in range(NCORES):
        y[c // 4] += res.results[c]["yT"].astype(np.float32).T
    return y


# revision 21
# speedup vs baseline: 1.2571x; 1.0081x over previous
"""Trainium2 Bass kernel for nn_MultiHeadedAttention_71425306132929.

Fused QKV projection + RoPE + causal/padding-masked SDPA + output projection.

Sharding: 8 cores = 2 batches x 4 head-groups (4 heads each).  Each core
computes, for its (batch, head-group):
    qkT = (Wq|Wk) @ query[b].T      (transposed layout: head-dim on partitions)
    RoPE on qT/kT via in-quadrant partition shuffle (head dims permuted
    host-side so RoPE partners are 16 partitions apart)
    scoresT[k,q] = kT.T-dot-qT per head (2 heads packed via PE row tiling)
    PT = exp(scoresT * 1/8)  (no max-subtraction needed: logits are O(1))
    causal masking: block-skip + trimmed exp/matmul APs + affine_select on
    the diagonal 128-blocks
    padding mask: folded into v (zeroed rows) + an extra all-mask column that
    makes the attention-denominator fall out of the same matmul
    ohT = (v|m).T @ PT accumulated over key blocks -> unnormalized out + denom
    normalize via reciprocal on a DMA-partition-packed view + broadcast loads
    yT_partial = WoutT.T @ ohT  (row-parallel out-projection, bf16 store)
Host sums the 4 partial yT per batch.

Scheduling: the attention stream is ACT(exp)-paced; emission order software-
pipelines scores one block ahead of PV and interleaves the next q-chunk's
projection matmuls + the previous chunk's out-projection as PE filler fed
from a generator queue.  At round boundaries only the two qk chains the next
round's first scores need are drained; the rest keeps filling.
"""

import os
import sys
from collections import deque

import numpy as np

sys.path.insert(0, "/opt/trn_rl_repo")

import concourse.bass as bass  # noqa: E402
import concourse.bacc as bacc  # noqa: E402
import concourse.tile as tile  # noqa: E402
from concourse import mybir  # noqa: E402

import ml_dtypes  # noqa: E402

BF16 = mybir.dt.bfloat16
F32 = mybir.dt.float32

B, S, DM, TD, H, HD = 2, 2048, 1024, 1024, 16, 64
NCORES = 8
NH = 4          # heads per core
NKB = S // 128  # 16 key blocks
NQC = S // 512  # 4 query chunks
KC = DM // 128  # 8 contraction chunks

# RoPE partner permutation: place original dim d so that partner(p) = p ^ 16
# (within a 32-partition quadrant, reachable by DVE stream_shuffle).
ROPE_PERM = []
for _p in range(64):
    q32, r32 = _p // 32, _p % 32
    ROPE_PERM.append(q32 * 16 + r32 if r32 < 16 else 32 + q32 * 16 + (r32 - 16))
ROPE_SGN = np.array([-1.0 if (p % 32) < 16 else 1.0 for p in range(64)], np.float32)
SHUF_MASK = [i ^ 16 for i in range(32)]

_CACHED = {}


class Gen:
    """Steppable wrapper over an emission generator."""

    def __init__(self, g):
        self.g = g
        self.done = False

    def step(self):
        if self.done:
            return False
        try:
            next(self.g)
            return True
        except StopIteration:
            self.done = True
            return False


def build_program(nvb=NKB):
    nc = bacc.Bacc(None, target_bir_lowering=False)
    qT_d = nc.declare_dram_parameter("qT", [DM, S], BF16, isOutput=False)
    wqk_d = nc.declare_dram_parameter("wqkT", [DM, 512], BF16, isOutput=False)
    wv_d = nc.declare_dram_parameter("wvT", [DM, 256], BF16, isOutput=False)
    cos_d = nc.declare_dram_parameter("cosT", [128, S], BF16, isOutput=False)
    sin_d = nc.declare_dram_parameter("sinT", [128, S], BF16, isOutput=False)
    mkv_d = nc.declare_dram_parameter("maskv", [128, NKB], F32, isOutput=False)
    wo_d = nc.declare_dram_parameter("woutT", [256, DM], BF16, isOutput=False)
    yT_d = nc.declare_dram_parameter("yT", [DM, S], BF16, isOutput=True)
    dscr2 = nc.dram_tensor("rcp_scratch", [16, 512], F32)

    with tile.TileContext(nc) as tc:
        with (
            tc.tile_pool(name="const", bufs=1) as cpool,
            tc.tile_pool(name="work", bufs=1) as wpool,
            tc.tile_pool(name="rope", bufs=3) as rpool,
            tc.tile_pool(name="pt", bufs=6) as ptpool,
            tc.tile_pool(name="nrm", bufs=4) as npool,
            tc.tile_pool(name="yout", bufs=2) as ypool,
            tc.tile_pool(name="psA", bufs=2, space="PSUM") as psA,
            tc.tile_pool(name="psP", bufs=2, space="PSUM") as psP,
            tc.tile_pool(name="psO", bufs=2, space="PSUM") as psO,
        ):
            qT_sb = cpool.tile([128, KC, S], BF16, tag="qT")
            wqk_sb = cpool.tile([128, KC, 512], BF16, tag="wqk")
            wv_sb = cpool.tile([128, KC, 256], BF16, tag="wv")
            cos_sb = cpool.tile([128, S], BF16, tag="cos")
            sin_sb = cpool.tile([128, S], BF16, tag="sin")
            mkv_sb = cpool.tile([128, NKB], F32, tag="mkv")
            wo_sb = cpool.tile([128, 2, DM], BF16, tag="wo")

            qk_sb = wpool.tile([128, 4, S], BF16, tag="qk")
            vaug_sb = wpool.tile([128, NKB, 4, 128], BF16, tag="vaug")
            ohT_sb = wpool.tile([128, 2, S], BF16, tag="ohT")

            wqk_r = wqk_d.rearrange("(c p) s -> p c s", p=128)
            qT_r = qT_d.rearrange("(c p) s -> p c s", p=128)
            wv_r = wv_d.rearrange("(c p) s -> p c s", p=128)
            wo_r = wo_d.rearrange("(c p) s -> p c s", p=128)
            yT_r = yT_d.rearrange("(c p) s -> p c s", p=128)

            # Input DMA, merged into few descriptors, ordered so q-chunk
            # 0's projections can start ASAP.
            nc.sync.dma_start(mkv_sb[:], mkv_d[:])
            qsl0 = slice(0, 512)
            # first q-chunk's weights/activations land per-kc so the first
            # projection chains can chase the arriving chunks
            for kc in range(KC):
                nc.sync.dma_start(wqk_sb[:, kc, :], wqk_r[:, kc, :])
                nc.sync.dma_start(qT_sb[:, kc, qsl0], qT_r[:, kc, qsl0])
            nc.sync.dma_start(wv_sb[:], wv_r[:])
            nc.sync.dma_start(cos_sb[:], cos_d[:])
            nc.sync.dma_start(sin_sb[:], sin_d[:])
            for qn in range(1, NQC):
                qsl = slice(qn * 512, qn * 512 + 512)
                nc.sync.dma_start(qT_sb[:, :, qsl], qT_r[:, :, qsl])
            nc.sync.dma_start(wo_sb[:], wo_r[:])

            # HAM warm-up: dependency-free matmuls keep the PE activity
            # monitor busy from t~=6us so the real prologue runs at 2.4GHz.
            warm_sb = cpool.tile([128, 128], BF16, tag="warm")
            nc.gpsimd.memset(warm_sb[:], 0.0)
            warm_ps = psP.tile([128, 128], F32, tag="psP", name="warm_ps")
            for _w in range(96):
                nc.tensor.matmul(warm_ps[:], lhsT=warm_sb[:], rhs=warm_sb[:],
                                 start=True, stop=True, skip_group_check=True)

            nc.gpsimd.memset(vaug_sb[:], 0.0)
            # mask columns of v_aug: even slots col 64, odd slots col 32
            # (den must land on a legal engine start partition: 0/32/64/96)
            mkv_col = mkv_sb.rearrange("p (k o) -> p k o", o=1)
            nc.gpsimd.tensor_copy(vaug_sb[:, :, 0, 64:65], mkv_col)
            nc.gpsimd.tensor_copy(vaug_sb[:, :, 2, 64:65], mkv_col)
            nc.gpsimd.tensor_copy(vaug_sb[:, :, 1, 32:33], mkv_col)
            nc.gpsimd.tensor_copy(vaug_sb[:, :, 3, 32:33], mkv_col)

            def emit_qk(mt, qn):
                """project + rope one [128, 512] chunk of q or k (pair of heads).

                Generator: yields after each matmul so the caller can
                interleave; RoPE tail (DVE shuffle/mul + GPS mul/add) on close.
                """
                qsl = slice(qn * 512, qn * 512 + 512)
                ps = psP.tile([128, 512], F32, tag="psP")
                for kc in range(KC):
                    nc.tensor.matmul(
                        ps[:],
                        lhsT=wqk_sb[:, kc, mt * 128:(mt + 1) * 128],
                        rhs=qT_sb[:, kc, qsl],
                        start=(kc == 0),
                        stop=(kc == KC - 1),
                    )
                    if kc == 3:
                        yield
                qkp = rpool.tile([128, 512], BF16, tag="qkp")
                nc.vector.tensor_copy(qkp[:], ps[:])
                shuf = rpool.tile([128, 512], BF16, tag="shuf")
                nc.vector.stream_shuffle(shuf[:], qkp[:], mask=SHUF_MASK)
                t1 = rpool.tile([128, 512], BF16, tag="t1")
                nc.vector.tensor_mul(t1[:], qkp[:], cos_sb[:, qsl])
                t2 = rpool.tile([128, 512], BF16, tag="t2")
                nc.vector.tensor_mul(t2[:], shuf[:], sin_sb[:, qsl])
                nc.vector.tensor_add(qk_sb[:, mt, qsl], t1[:], t2[:])
                yield

            def emit_v(st):
                """project + mask one [128 keys, 4*64] v block into v_aug."""
                ps = psP.tile([128, 512], F32, tag="psP")
                psv = ps[:, 0:256]
                for kc in range(KC):
                    nc.tensor.matmul(
                        psv,
                        lhsT=qT_sb[:, kc, st * 128:(st + 1) * 128],
                        rhs=wv_sb[:, kc, :],
                        start=(kc == 0),
                        stop=(kc == KC - 1),
                    )
                    if kc == 3:
                        yield
                psv_h = psv.rearrange("p (h d) -> p h d", h=4)
                msk = mkv_sb[:, st:st + 1]
                # even local heads (slots 0,2) -> cols 0:64 ; odd -> cols 64:128
                nc.vector.tensor_scalar_mul(
                    vaug_sb[:, st, 0:4:2, 0:64], psv_h[:, 0:4:2, :], msk)
                nc.vector.tensor_scalar_mul(
                    vaug_sb[:, st, 1:4:2, 64:128], psv_h[:, 1:4:2, :], msk)
                yield

            def emit_outproj(qn):
                """row-parallel out-projection of one q chunk (bf16 store)."""
                qsl = slice(qn * 512, qn * 512 + 512)
                y = ypool.tile([128, 8, 512], BF16, tag="y")
                for mt in range(8):
                    ps = psP.tile([128, 512], F32, tag="psP")
                    for kc2 in range(2):
                        nc.tensor.matmul(
                            ps[:],
                            lhsT=wo_sb[:, kc2, mt * 128:(mt + 1) * 128],
                            rhs=ohT_sb[:, kc2, qsl],
                            start=(kc2 == 0),
                            stop=(kc2 == 1),
                        )
                    nc.any.tensor_copy(y[:, mt, :], ps[:])
                    if mt % 2 == 1:
                        nc.sync.dma_start(yT_r[:, mt - 1:mt + 1, qsl],
                                          y[:, mt - 1:mt + 1, :])
                    yield

            def emit_outproj_A(qn, y32):
                """pair-0 half of the last out-projection: runs during the
                final pair's attention, result parked in f32 SBUF."""
                qsl = slice(qn * 512, qn * 512 + 512)
                for mt in range(8):
                    ps = psP.tile([128, 512], F32, tag="psP")
                    nc.tensor.matmul(
                        ps[:],
                        lhsT=wo_sb[:, 0, mt * 128:(mt + 1) * 128],
                        rhs=ohT_sb[:, 0, qsl],
                        start=True, stop=True,
                    )
                    nc.any.tensor_copy(y32[:, mt, :], ps[:])
                    yield

            def emit_outproj_B(qn, y32):
                """pair-1 half + combine + store (the only post-norm work)."""
                qsl = slice(qn * 512, qn * 512 + 512)
                y = ypool.tile([128, 8, 512], BF16, tag="y")
                for mt in range(8):
                    ps = psP.tile([128, 512], F32, tag="psP")
                    nc.tensor.matmul(
                        ps[:],
                        lhsT=wo_sb[:, 1, mt * 128:(mt + 1) * 128],
                        rhs=ohT_sb[:, 1, qsl],
                        start=True, stop=True,
                    )
                    nc.vector.tensor_add(y[:, mt, :], y32[:, mt, :], ps[:])
                    if mt % 2 == 1:
                        nc.sync.dma_start(yT_r[:, mt - 1:mt + 1, qsl],
                                          y[:, mt - 1:mt + 1, :])
                    yield

            # ---- filler queue: projection / out-projection emission units
            # interleaved into the ACT-paced attention stream ----
            filler = deque()

            def pump(n):
                done = 0
                while done < n and filler:
                    if filler[0].step():
                        done += 1
                    else:
                        filler.popleft()

            def drain(*gens):
                for g in gens:
                    while g.step():
                        pass

            def flush():
                while filler:
                    if not filler[0].step():
                        filler.popleft()

            def queue_prep(qc):
                """queue next q-chunk's projections; returns handles:
                (pair0 qk chains, v chains, pair1 qk chains).  k/v chains
                past the padding-valid key range are never needed."""
                p0 = [Gen(emit_qk(0, qc))]
                p1 = [Gen(emit_qk(1, qc))]
                if qc * 512 < nvb * 128:
                    p0.insert(0, Gen(emit_qk(2, qc)))
                    p1.insert(0, Gen(emit_qk(3, qc)))
                vs = [Gen(emit_v(st))
                      for st in range(4 * qc, 4 * qc + 4) if st < nvb]
                for g in p0 + vs + p1:
                    filler.append(g)
                return p0, vs, p1

            def emit_attn(pair, qc, v_gens=None):
                """ACT-paced attention for one head-pair and q chunk.

                Software pipeline: scores one block ahead of PV; exp and
                matmul APs trimmed to the causally-valid q columns.
                v_gens: this chunk's diagonal v-projection chains — drained
                just before the first diagonal block's PV can need them.
                """
                nkb = min(4 * qc + 4, nvb)
                qmt, kmt = pair, 2 + pair
                oT = [psO.tile([128, 512], F32, tag="psO", name=f"oT{_h}")
                      for _h in range(2)]
                pts = [None] * nkb   # (pt tile, co) per block
                for kb in range(nkb):
                    if kb == max(4 * qc - 1, 0) and v_gens:
                        drain(*v_gens)
                    ksl = slice(kb * 128, kb * 128 + 128)
                    joff = kb - 4 * qc
                    co = max(joff, 0) * 128  # first causally-valid q col
                    st_ps = psA.tile([128, 1024], F32, tag="psA", name="stps")
                    for h in range(2):
                        pr = slice(64 * h, 64 * h + 64)
                        nc.tensor.matmul(
                            st_ps[:, h * 512 + co:(h + 1) * 512],
                            lhsT=qk_sb[pr, kmt, ksl],
                            rhs=qk_sb[pr, qmt, qc * 512 + co:qc * 512 + 512],
                            start=True,
                            stop=True,
                            skip_group_check=True,
                        )
                    pt = ptpool.tile([128, 1024], BF16, tag="pt", name="pt")
                    pts[kb] = (pt, co)
                    if co == 0:
                        nc.scalar.activation(
                            pt[:], st_ps[:],
                            mybir.ActivationFunctionType.Exp,
                            scale=0.125,
                        )
                    else:
                        # strided 2-range AP: only the valid q cols per head
                        pt_v = pt.rearrange("p (h q) -> p h q", h=2)
                        st_v = st_ps.rearrange("p (h q) -> p h q", h=2)
                        nc.scalar.activation(
                            pt_v[:, :, co:512], st_v[:, :, co:512],
                            mybir.ActivationFunctionType.Exp,
                            scale=0.125,
                        )
                    if joff >= 0:
                        for h in range(2):
                            nc.gpsimd.affine_select(
                                pt[:, h * 512 + co:h * 512 + co + 128],
                                pt[:, h * 512 + co:h * 512 + co + 128],
                                pattern=[[1, 128]],
                                compare_op=mybir.AluOpType.is_ge,
                                fill=0.0,
                                base=0,
                                channel_multiplier=-1,
                            )
                    if kb >= 1:
                        emit_pv(pair, kb - 1, pts[kb - 1], oT, nkb)
                    pump(1)
                emit_pv(pair, nkb - 1, pts[nkb - 1], oT, nkb)
                emit_norm(pair, qc, oT)

            def emit_pv(pair, kb, pt_co, oT, nkb):
                pt, co = pt_co
                for h in range(2):
                    nc.tensor.matmul(
                        oT[h][:, co:512],
                        lhsT=vaug_sb[:, kb, 2 * pair + h, :],
                        rhs=pt[:, h * 512 + co:(h + 1) * 512],
                        start=(kb == 0),
                        stop=(kb == nkb - 1),
                        skip_group_check=True,
                    )

            def emit_norm_fast(pair, qc, oT):
                """final-norm variant: SBUF pack -> reciprocal -> unpack ->
                GPSIMD partition_broadcast.  No DRAM bounce, single-packet
                DMAs — this chain is exposed at the kernel tail."""
                qsl = slice(qc * 512, qc * 512 + 512)
                osb = []
                for h in range(2):
                    o = npool.tile([128, 512], F32, tag="osb", name=f"osb{h}")
                    nc.vector.tensor_copy(o[:], oT[h][:])
                    osb.append(o)
                rcp = npool.tile([128, 8], F32, tag="rcp")
                nc.sync.dma_start(rcp[0:64, :], osb[0][64:65, :],
                                  single_packet=True)
                nc.sync.dma_start(rcp[64:128, :], osb[1][32:33, :],
                                  single_packet=True)
                rcp2 = npool.tile([128, 8], F32, tag="rcp2")
                nc.vector.reciprocal(rcp2[:], rcp[:])
                rowA = npool.tile([128, 512], F32, tag="rowA")
                rowB = npool.tile([128, 512], F32, tag="rowB")
                nc.sync.dma_start(rowA[0:1, :], rcp2[0:64, :],
                                  single_packet=True)
                nc.sync.dma_start(rowB[0:1, :], rcp2[64:128, :],
                                  single_packet=True)
                bc = npool.tile([128, 512], F32, tag="bc")
                nc.gpsimd.partition_broadcast(bc[0:64, :], rowA[0:1, :])
                nc.gpsimd.partition_broadcast(bc[64:128, :], rowB[0:1, :])
                nc.vector.tensor_mul(
                    ohT_sb[0:64, pair, qsl], osb[0][0:64, :], bc[0:64, :])
                nc.vector.tensor_mul(
                    ohT_sb[64:128, pair, qsl], osb[1][64:128, :], bc[64:128, :])

            def emit_norm(pair, qc, oT):
                """normalize: SBUF->SBUF DMA packs the two [1,512] den rows
                into [128,8] lanes for a cheap reciprocal, then a DRAM
                bounce provides the partition-broadcast.  No PE instruction
                in this chain: the in-order PE stream must never block on
                DMA latency."""
                qsl = slice(qc * 512, qc * 512 + 512)
                base = (pair * 4 + qc) * 2
                osb = []
                for h in range(2):
                    o = npool.tile([128, 512], F32, tag="osb", name=f"osb{h}")
                    nc.vector.tensor_copy(o[:], oT[h][:])  # frees the psum bank
                    osb.append(o)
                rcp = npool.tile([128, 8], F32, tag="rcp")
                nc.sync.dma_start(rcp[0:64, :], osb[0][64:65, :])
                nc.sync.dma_start(rcp[64:128, :], osb[1][32:33, :])
                rcp2 = npool.tile([128, 8], F32, tag="rcp2")
                nc.vector.reciprocal(rcp2[:], rcp[:])
                nc.sync.dma_start(
                    dscr2[base:base + 2, :].rearrange("a (p f) -> (a p) f", f=8), rcp2[:])
                bc = npool.tile([128, 512], F32, tag="bc")
                nc.gpsimd.dma_start(bc[0:64, :],
                                    dscr2[base:base + 1, :].to_broadcast((64, 512)))
                nc.gpsimd.dma_start(bc[64:128, :],
                                    dscr2[base + 1:base + 2, :].to_broadcast((64, 512)))
                nc.vector.tensor_mul(
                    ohT_sb[0:64, pair, qsl], osb[0][0:64, :], bc[0:64, :])
                nc.vector.tensor_mul(
                    ohT_sb[64:128, pair, qsl], osb[1][64:128, :], bc[64:128, :])

            # ---- main schedule ----
            # prologue: only the two qk chains attn(0,0) needs, drained;
            # the rest of prep(0) goes through the filler queue.
            g20 = Gen(emit_qk(2, 0))
            g00 = Gen(emit_qk(0, 0))
            drain(g20, g00)
            vs = [Gen(emit_v(st)) for st in range(4)]
            p1 = [Gen(emit_qk(3, 0)), Gen(emit_qk(1, 0))]
            for g in vs + p1:
                filler.append(g)

            y32 = wpool.tile([128, 8, 512], F32, tag="y32")
            for qc in range(NQC):
                nxt = queue_prep(qc + 1) if qc + 1 < NQC else None
                emit_attn(0, qc, vs)
                if qc == NQC - 1:
                    filler.append(Gen(emit_outproj_A(qc, y32)))
                drain(*p1)
                emit_attn(1, qc, vs)
                if qc == NQC - 1:
                    flush()
                    for _ in emit_outproj_B(qc, y32):
                        pass
                else:
                    filler.append(Gen(emit_outproj(qc)))
                if nxt is not None:
                    p0, vs, p1 = nxt
                    drain(*p0)  # the two chains attn(0, qc+1) starts on
            flush()

    nc.compile()
    return nc


def make_in_maps(query, W_in, W_out, sin_q, cos_q, attn_mask):
    bf = ml_dtypes.bfloat16
    cosT = np.asarray(cos_q, np.float32)[0, 0].T  # [64, S]
    sinT = np.asarray(sin_q, np.float32)[0, 0].T
    cosT_p = cosT[ROPE_PERM]
    sinT_p = sinT[ROPE_PERM] * ROPE_SGN[:, None]
    cos2 = np.concatenate([cosT_p, cosT_p], 0).astype(bf)    # [128, S]
    sin2 = np.concatenate([sinT_p, sinT_p], 0).astype(bf)
    W_in = np.asarray(W_in, np.float32)
    W_out = np.asarray(W_out, np.float32)
    query = np.asarray(query, np.float32)
    attn_mask = np.asarray(attn_mask)

    in_maps = []
    for c in range(NCORES):
        b, g = c // 4, c % 4
        heads = range(4 * g, 4 * g + 4)
        qrows = np.concatenate([W_in[h * 64:(h + 1) * 64][ROPE_PERM] for h in heads])
        krows = np.concatenate([W_in[TD + h * 64:TD + (h + 1) * 64][ROPE_PERM] for h in heads])
        vrows = np.concatenate([W_in[2 * TD + h * 64:2 * TD + (h + 1) * 64] for h in heads])
        tcols = np.concatenate([np.arange(h * 64, (h + 1) * 64) for h in heads])
        in_maps.append({
            "qT": np.ascontiguousarray(query[b].T).astype(bf),
            "wqkT": np.ascontiguousarray(np.concatenate([qrows, krows], 0).T).astype(bf),
            "wvT": np.ascontiguousarray(vrows.T).astype(bf),
            "cosT": cos2,
            "sinT": sin2,
            "maskv": np.ascontiguousarray(
                attn_mask[b].astype(np.float32).reshape(NKB, 128).T),
            "woutT": np.ascontiguousarray(W_out[:, tcols].T).astype(bf),
        })
    return in_maps


def _ensure_ntff_hook():
    """The image's antenv lacks axon_hooks; supply it so trace=True works."""
    try:
        from antenv.axon_hooks import get_axon_ntff_profile_hook  # noqa: F401
        return
    except ImportError:
        pass
    import types

    if "/root/.axon_site" not in sys.path:
        sys.path.insert(0, "/root/.axon_site")
    from trn_agent_boot.trn_boot import _ntff_profile_via_ctypes

    hook = _ntff_profile_via_ctypes("/opt/axon/libaxon_pjrt.so")
    mod = types.ModuleType("antenv.axon_hooks")
    mod._hook = hook
    mod.get_axon_ntff_profile_hook = lambda: mod._hook
    mod.set_axon_ntff_profile_hook = lambda h: setattr(mod, "_hook", h)
    sys.modules["antenv.axon_hooks"] = mod
    import antenv

    antenv.axon_hooks = mod


def kernel(query, W_in, W_out, sin_q, cos_q, attn_mask):
    mask = np.asarray(attn_mask)
    nvb = 1
    for b in range(B):
        idx = np.nonzero(mask[b])[0]
        last = int(idx[-1]) if idx.size else 0
        nvb = max(nvb, last // 128 + 1)
    key = ("nc", nvb)
    if key not in _CACHED:
        _CACHED[key] = build_program(nvb)
    nc = _CACHED[key]
    in_maps = make_in_maps(query, W_in, W_out, sin_q, cos_q, attn_mask)

    from concourse.bass_utils import run_bass_kernel_spmd

    trace = bool(os.environ.get("KERNEL_PROFILE"))
    if trace:
        try:
            _ensure_ntff_hook()
        except Exception as e:  # profiling is best-effort
            print(f"ntff hook unavailable: {e}")
            trace = False
    try:
        res = run_bass_kernel_spmd(nc, in_maps, list(range(NCORES)), trace=trace)
    except Exception:
        if not trace:
            raise
        res = run_bass_kernel_spmd(nc, in_maps, list(range(NCORES)), trace=False)
    _CACHED["last_result"] = res

    y = np.zeros((B, S, DM), np.float32)
    for c in range(NCORES):
        y[c // 4] += res.results[c]["yT"].astype(np.float32).T
    return y


# revision 22
# speedup vs baseline: 1.2686x; 1.0091x over previous
"""Trainium2 Bass kernel for nn_MultiHeadedAttention_71425306132929.

Fused QKV projection + RoPE + causal/padding-masked SDPA + output projection.

Sharding: 8 cores = 2 batches x 4 head-groups (4 heads each).  Each core
computes, for its (batch, head-group):
    qkT = (Wq|Wk) @ query[b].T      (transposed layout: head-dim on partitions)
    RoPE on qT/kT via in-quadrant partition shuffle (head dims permuted
    host-side so RoPE partners are 16 partitions apart)
    scoresT[k,q] = kT.T-dot-qT per head (2 heads packed via PE row tiling)
    PT = exp(scoresT * 1/8)  (no max-subtraction needed: logits are O(1))
    causal masking: block-skip + trimmed exp/matmul APs + affine_select on
    the diagonal 128-blocks
    padding mask: folded into v (zeroed rows) + an extra all-mask column that
    makes the attention-denominator fall out of the same matmul
    ohT = (v|m).T @ PT accumulated over key blocks -> unnormalized out + denom
    normalize via reciprocal on a DMA-partition-packed view + broadcast loads
    yT_partial = WoutT.T @ ohT  (row-parallel out-projection, bf16 store)
Host sums the 4 partial yT per batch.

Scheduling: the attention stream is ACT(exp)-paced; emission order software-
pipelines scores one block ahead of PV and interleaves the next q-chunk's
projection matmuls + the previous chunk's out-projection as PE filler fed
from a generator queue.  At round boundaries only the two qk chains the next
round's first scores need are drained; the rest keeps filling.
"""

import os
import sys
from collections import deque

import numpy as np

sys.path.insert(0, "/opt/trn_rl_repo")

import concourse.bass as bass  # noqa: E402
import concourse.bacc as bacc  # noqa: E402
import concourse.tile as tile  # noqa: E402
from concourse import mybir  # noqa: E402

import ml_dtypes  # noqa: E402

BF16 = mybir.dt.bfloat16
F32 = mybir.dt.float32

B, S, DM, TD, H, HD = 2, 2048, 1024, 1024, 16, 64
NCORES = 8
NH = 4          # heads per core
NKB = S // 128  # 16 key blocks
NQC = S // 512  # 4 query chunks
KC = DM // 128  # 8 contraction chunks

# RoPE partner permutation: place original dim d so that partner(p) = p ^ 16
# (within a 32-partition quadrant, reachable by DVE stream_shuffle).
ROPE_PERM = []
for _p in range(64):
    q32, r32 = _p // 32, _p % 32
    ROPE_PERM.append(q32 * 16 + r32 if r32 < 16 else 32 + q32 * 16 + (r32 - 16))
ROPE_SGN = np.array([-1.0 if (p % 32) < 16 else 1.0 for p in range(64)], np.float32)
SHUF_MASK = [i ^ 16 for i in range(32)]

_CACHED = {}


class Gen:
    """Steppable wrapper over an emission generator."""

    def __init__(self, g):
        self.g = g
        self.done = False

    def step(self):
        if self.done:
            return False
        try:
            next(self.g)
            return True
        except StopIteration:
            self.done = True
            return False


def build_program(nvb=NKB):
    nc = bacc.Bacc(None, target_bir_lowering=False)
    qT_d = nc.declare_dram_parameter("qT", [DM, S], BF16, isOutput=False)
    wqk_d = nc.declare_dram_parameter("wqkT", [DM, 512], BF16, isOutput=False)
    wv_d = nc.declare_dram_parameter("wvT", [DM, 256], BF16, isOutput=False)
    cos_d = nc.declare_dram_parameter("cosT", [128, S], BF16, isOutput=False)
    sin_d = nc.declare_dram_parameter("sinT", [128, S], BF16, isOutput=False)
    mkv_d = nc.declare_dram_parameter("maskv", [128, NKB], F32, isOutput=False)
    wo_d = nc.declare_dram_parameter("woutT", [256, DM], BF16, isOutput=False)
    yT_d = nc.declare_dram_parameter("yT", [DM, S], BF16, isOutput=True)
    dscr2 = nc.dram_tensor("rcp_scratch", [16, 512], F32)

    with tile.TileContext(nc) as tc:
        with (
            tc.tile_pool(name="const", bufs=1) as cpool,
            tc.tile_pool(name="work", bufs=1) as wpool,
            tc.tile_pool(name="rope", bufs=4) as rpool,
            tc.tile_pool(name="pt", bufs=8) as ptpool,
            tc.tile_pool(name="nrm", bufs=4) as npool,
            tc.tile_pool(name="yout", bufs=2) as ypool,
            tc.tile_pool(name="psA", bufs=2, space="PSUM") as psA,
            tc.tile_pool(name="psP", bufs=2, space="PSUM") as psP,
            tc.tile_pool(name="psO", bufs=2, space="PSUM") as psO,
        ):
            qT_sb = cpool.tile([128, KC, S], BF16, tag="qT")
            wqk_sb = cpool.tile([128, KC, 512], BF16, tag="wqk")
            wv_sb = cpool.tile([128, KC, 256], BF16, tag="wv")
            cos_sb = cpool.tile([128, S], BF16, tag="cos")
            sin_sb = cpool.tile([128, S], BF16, tag="sin")
            mkv_sb = cpool.tile([128, NKB], F32, tag="mkv")
            wo_sb = cpool.tile([128, 2, DM], BF16, tag="wo")

            qk_sb = wpool.tile([128, 4, S], BF16, tag="qk")
            vaug_sb = wpool.tile([128, NKB, 4, 128], BF16, tag="vaug")
            ohT_sb = wpool.tile([128, 2, S], BF16, tag="ohT")

            wqk_r = wqk_d.rearrange("(c p) s -> p c s", p=128)
            qT_r = qT_d.rearrange("(c p) s -> p c s", p=128)
            wv_r = wv_d.rearrange("(c p) s -> p c s", p=128)
            wo_r = wo_d.rearrange("(c p) s -> p c s", p=128)
            yT_r = yT_d.rearrange("(c p) s -> p c s", p=128)

            # Input DMA, merged into few descriptors, ordered so q-chunk
            # 0's projections can start ASAP.
            nc.sync.dma_start(mkv_sb[:], mkv_d[:])
            qsl0 = slice(0, 512)
            # first q-chunk's weights/activations land per-kc so the first
            # projection chains can chase the arriving chunks
            for kc in range(KC):
                nc.sync.dma_start(wqk_sb[:, kc, :], wqk_r[:, kc, :])
                nc.sync.dma_start(qT_sb[:, kc, qsl0], qT_r[:, kc, qsl0])
            nc.sync.dma_start(wv_sb[:], wv_r[:])
            nc.sync.dma_start(cos_sb[:], cos_d[:])
            nc.sync.dma_start(sin_sb[:], sin_d[:])
            for qn in range(1, NQC):
                qsl = slice(qn * 512, qn * 512 + 512)
                nc.sync.dma_start(qT_sb[:, :, qsl], qT_r[:, :, qsl])
            nc.sync.dma_start(wo_sb[:], wo_r[:])

            # HAM warm-up: dependency-free matmuls keep the PE activity
            # monitor busy from t~=6us so the real prologue runs at 2.4GHz.
            warm_sb = cpool.tile([128, 128], BF16, tag="warm")
            nc.gpsimd.memset(warm_sb[:], 0.0)
            warm_ps = psP.tile([128, 128], F32, tag="psP", name="warm_ps")
            for _w in range(96):
                nc.tensor.matmul(warm_ps[:], lhsT=warm_sb[:], rhs=warm_sb[:],
                                 start=True, stop=True, skip_group_check=True)

            nc.gpsimd.memset(vaug_sb[:], 0.0)
            # mask columns of v_aug: even slots col 64, odd slots col 32
            # (den must land on a legal engine start partition: 0/32/64/96)
            mkv_col = mkv_sb.rearrange("p (k o) -> p k o", o=1)
            nc.gpsimd.tensor_copy(vaug_sb[:, :, 0, 64:65], mkv_col)
            nc.gpsimd.tensor_copy(vaug_sb[:, :, 2, 64:65], mkv_col)
            nc.gpsimd.tensor_copy(vaug_sb[:, :, 1, 32:33], mkv_col)
            nc.gpsimd.tensor_copy(vaug_sb[:, :, 3, 32:33], mkv_col)

            def emit_qk(mt, qn):
                """project + rope one [128, 512] chunk of q or k (pair of heads).

                Generator: yields after each matmul so the caller can
                interleave; RoPE tail (DVE shuffle/mul + GPS mul/add) on close.
                """
                qsl = slice(qn * 512, qn * 512 + 512)
                ps = psP.tile([128, 512], F32, tag="psP")
                for kc in range(KC):
                    nc.tensor.matmul(
                        ps[:],
                        lhsT=wqk_sb[:, kc, mt * 128:(mt + 1) * 128],
                        rhs=qT_sb[:, kc, qsl],
                        start=(kc == 0),
                        stop=(kc == KC - 1),
                    )
                    if kc == 3:
                        yield
                qkp = rpool.tile([128, 512], BF16, tag="qkp")
                nc.vector.tensor_copy(qkp[:], ps[:])
                shuf = rpool.tile([128, 512], BF16, tag="shuf")
                nc.vector.stream_shuffle(shuf[:], qkp[:], mask=SHUF_MASK)
                t1 = rpool.tile([128, 512], BF16, tag="t1")
                nc.vector.tensor_mul(t1[:], qkp[:], cos_sb[:, qsl])
                t2 = rpool.tile([128, 512], BF16, tag="t2")
                nc.vector.tensor_mul(t2[:], shuf[:], sin_sb[:, qsl])
                nc.vector.tensor_add(qk_sb[:, mt, qsl], t1[:], t2[:])
                yield

            def emit_v(st):
                """project + mask one [128 keys, 4*64] v block into v_aug."""
                ps = psP.tile([128, 512], F32, tag="psP")
                psv = ps[:, 0:256]
                for kc in range(KC):
                    nc.tensor.matmul(
                        psv,
                        lhsT=qT_sb[:, kc, st * 128:(st + 1) * 128],
                        rhs=wv_sb[:, kc, :],
                        start=(kc == 0),
                        stop=(kc == KC - 1),
                    )
                    if kc == 3:
                        yield
                psv_h = psv.rearrange("p (h d) -> p h d", h=4)
                msk = mkv_sb[:, st:st + 1]
                # even local heads (slots 0,2) -> cols 0:64 ; odd -> cols 64:128
                nc.vector.tensor_scalar_mul(
                    vaug_sb[:, st, 0:4:2, 0:64], psv_h[:, 0:4:2, :], msk)
                nc.vector.tensor_scalar_mul(
                    vaug_sb[:, st, 1:4:2, 64:128], psv_h[:, 1:4:2, :], msk)
                yield

            def emit_outproj(qn):
                """row-parallel out-projection of one q chunk (bf16 store)."""
                qsl = slice(qn * 512, qn * 512 + 512)
                y = ypool.tile([128, 8, 512], BF16, tag="y")
                for mt in range(8):
                    ps = psP.tile([128, 512], F32, tag="psP")
                    for kc2 in range(2):
                        nc.tensor.matmul(
                            ps[:],
                            lhsT=wo_sb[:, kc2, mt * 128:(mt + 1) * 128],
                            rhs=ohT_sb[:, kc2, qsl],
                            start=(kc2 == 0),
                            stop=(kc2 == 1),
                        )
                    nc.any.tensor_copy(y[:, mt, :], ps[:])
                    if mt % 2 == 1:
                        nc.sync.dma_start(yT_r[:, mt - 1:mt + 1, qsl],
                                          y[:, mt - 1:mt + 1, :])
                    yield

            def emit_outproj_A(qn, y32):
                """pair-0 half of the last out-projection: runs during the
                final pair's attention, result parked in f32 SBUF."""
                qsl = slice(qn * 512, qn * 512 + 512)
                for mt in range(8):
                    ps = psP.tile([128, 512], F32, tag="psP")
                    nc.tensor.matmul(
                        ps[:],
                        lhsT=wo_sb[:, 0, mt * 128:(mt + 1) * 128],
                        rhs=ohT_sb[:, 0, qsl],
                        start=True, stop=True,
                    )
                    nc.any.tensor_copy(y32[:, mt, :], ps[:])
                    yield

            def emit_outproj_B(qn, y32):
                """pair-1 half + combine + store (the only post-norm work)."""
                qsl = slice(qn * 512, qn * 512 + 512)
                y = ypool.tile([128, 8, 512], BF16, tag="y")
                for mt in range(8):
                    ps = psP.tile([128, 512], F32, tag="psP")
                    nc.tensor.matmul(
                        ps[:],
                        lhsT=wo_sb[:, 1, mt * 128:(mt + 1) * 128],
                        rhs=ohT_sb[:, 1, qsl],
                        start=True, stop=True,
                    )
                    nc.vector.tensor_add(y[:, mt, :], y32[:, mt, :], ps[:])
                    if mt % 2 == 1:
                        nc.sync.dma_start(yT_r[:, mt - 1:mt + 1, qsl],
                                          y[:, mt - 1:mt + 1, :])
                    yield

            # ---- filler queue: projection / out-projection emission units
            # interleaved into the ACT-paced attention stream ----
            filler = deque()

            def pump(n):
                done = 0
                while done < n and filler:
                    if filler[0].step():
                        done += 1
                    else:
                        filler.popleft()

            def drain(*gens):
                for g in gens:
                    while g.step():
                        pass

            def flush():
                while filler:
                    if not filler[0].step():
                        filler.popleft()

            def queue_prep(qc):
                """queue next q-chunk's projections; returns handles:
                (pair0 qk chains, v chains, pair1 qk chains).  k/v chains
                past the padding-valid key range are never needed."""
                p0 = [Gen(emit_qk(0, qc))]
                p1 = [Gen(emit_qk(1, qc))]
                if qc * 512 < nvb * 128:
                    p0.insert(0, Gen(emit_qk(2, qc)))
                    p1.insert(0, Gen(emit_qk(3, qc)))
                vs = [Gen(emit_v(st))
                      for st in range(4 * qc, 4 * qc + 4) if st < nvb]
                for g in p0 + vs + p1:
                    filler.append(g)
                return p0, vs, p1

            def emit_attn(pair, qc, v_gens=None):
                """ACT-paced attention for one head-pair and q chunk.

                Software pipeline: scores one block ahead of PV; exp and
                matmul APs trimmed to the causally-valid q columns.
                v_gens: this chunk's diagonal v-projection chains — drained
                just before the first diagonal block's PV can need them.
                """
                nkb = min(4 * qc + 4, nvb)
                qmt, kmt = pair, 2 + pair
                oT = [psO.tile([128, 512], F32, tag="psO", name=f"oT{_h}")
                      for _h in range(2)]
                pts = [None] * nkb   # (pt tile, co) per block
                for kb in range(nkb):
                    if kb == max(4 * qc - 1, 0) and v_gens:
                        drain(*v_gens)
                    ksl = slice(kb * 128, kb * 128 + 128)
                    joff = kb - 4 * qc
                    co = max(joff, 0) * 128  # first causally-valid q col
                    st_ps = psA.tile([128, 1024], F32, tag="psA", name="stps")
                    for h in range(2):
                        pr = slice(64 * h, 64 * h + 64)
                        nc.tensor.matmul(
                            st_ps[:, h * 512 + co:(h + 1) * 512],
                            lhsT=qk_sb[pr, kmt, ksl],
                            rhs=qk_sb[pr, qmt, qc * 512 + co:qc * 512 + 512],
                            start=True,
                            stop=True,
                            skip_group_check=True,
                        )
                    pt = ptpool.tile([128, 1024], BF16, tag="pt", name="pt")
                    pts[kb] = (pt, co)
                    if co == 0:
                        nc.scalar.activation(
                            pt[:], st_ps[:],
                            mybir.ActivationFunctionType.Exp,
                            scale=0.125,
                        )
                    else:
                        # strided 2-range AP: only the valid q cols per head
                        pt_v = pt.rearrange("p (h q) -> p h q", h=2)
                        st_v = st_ps.rearrange("p (h q) -> p h q", h=2)
                        nc.scalar.activation(
                            pt_v[:, :, co:512], st_v[:, :, co:512],
                            mybir.ActivationFunctionType.Exp,
                            scale=0.125,
                        )
                    if joff >= 0:
                        for h in range(2):
                            nc.gpsimd.affine_select(
                                pt[:, h * 512 + co:h * 512 + co + 128],
                                pt[:, h * 512 + co:h * 512 + co + 128],
                                pattern=[[1, 128]],
                                compare_op=mybir.AluOpType.is_ge,
                                fill=0.0,
                                base=0,
                                channel_multiplier=-1,
                            )
                    if kb >= 1:
                        emit_pv(pair, kb - 1, pts[kb - 1], oT, nkb)
                    pump(1)
                emit_pv(pair, nkb - 1, pts[nkb - 1], oT, nkb)
                emit_norm(pair, qc, oT)

            def emit_pv(pair, kb, pt_co, oT, nkb):
                pt, co = pt_co
                for h in range(2):
                    nc.tensor.matmul(
                        oT[h][:, co:512],
                        lhsT=vaug_sb[:, kb, 2 * pair + h, :],
                        rhs=pt[:, h * 512 + co:(h + 1) * 512],
                        start=(kb == 0),
                        stop=(kb == nkb - 1),
                        skip_group_check=True,
                    )

            def emit_norm_fast(pair, qc, oT):
                """final-norm variant: SBUF pack -> reciprocal -> unpack ->
                GPSIMD partition_broadcast.  No DRAM bounce, single-packet
                DMAs — this chain is exposed at the kernel tail."""
                qsl = slice(qc * 512, qc * 512 + 512)
                osb = []
                for h in range(2):
                    o = npool.tile([128, 512], F32, tag="osb", name=f"osb{h}")
                    nc.vector.tensor_copy(o[:], oT[h][:])
                    osb.append(o)
                rcp = npool.tile([128, 8], F32, tag="rcp")
                nc.sync.dma_start(rcp[0:64, :], osb[0][64:65, :],
                                  single_packet=True)
                nc.sync.dma_start(rcp[64:128, :], osb[1][32:33, :],
                                  single_packet=True)
                rcp2 = npool.tile([128, 8], F32, tag="rcp2")
                nc.vector.reciprocal(rcp2[:], rcp[:])
                rowA = npool.tile([128, 512], F32, tag="rowA")
                rowB = npool.tile([128, 512], F32, tag="rowB")
                nc.sync.dma_start(rowA[0:1, :], rcp2[0:64, :],
                                  single_packet=True)
                nc.sync.dma_start(rowB[0:1, :], rcp2[64:128, :],
                                  single_packet=True)
                bc = npool.tile([128, 512], F32, tag="bc")
                nc.gpsimd.partition_broadcast(bc[0:64, :], rowA[0:1, :])
                nc.gpsimd.partition_broadcast(bc[64:128, :], rowB[0:1, :])
                nc.vector.tensor_mul(
                    ohT_sb[0:64, pair, qsl], osb[0][0:64, :], bc[0:64, :])
                nc.vector.tensor_mul(
                    ohT_sb[64:128, pair, qsl], osb[1][64:128, :], bc[64:128, :])

            def emit_norm(pair, qc, oT):
                """normalize: SBUF->SBUF DMA packs the two [1,512] den rows
                into [128,8] lanes for a cheap reciprocal, then a DRAM
                bounce provides the partition-broadcast.  No PE instruction
                in this chain: the in-order PE stream must never block on
                DMA latency."""
                qsl = slice(qc * 512, qc * 512 + 512)
                base = (pair * 4 + qc) * 2
                osb = []
                for h in range(2):
                    o = npool.tile([128, 512], F32, tag="osb", name=f"osb{h}")
                    nc.vector.tensor_copy(o[:], oT[h][:])  # frees the psum bank
                    osb.append(o)
                rcp = npool.tile([128, 8], F32, tag="rcp")
                nc.sync.dma_start(rcp[0:64, :], osb[0][64:65, :])
                nc.sync.dma_start(rcp[64:128, :], osb[1][32:33, :])
                rcp2 = npool.tile([128, 8], F32, tag="rcp2")
                nc.vector.reciprocal(rcp2[:], rcp[:])
                nc.sync.dma_start(
                    dscr2[base:base + 2, :].rearrange("a (p f) -> (a p) f", f=8), rcp2[:])
                bc = npool.tile([128, 512], F32, tag="bc")
                nc.gpsimd.dma_start(bc[0:64, :],
                                    dscr2[base:base + 1, :].to_broadcast((64, 512)))
                nc.gpsimd.dma_start(bc[64:128, :],
                                    dscr2[base + 1:base + 2, :].to_broadcast((64, 512)))
                nc.vector.tensor_mul(
                    ohT_sb[0:64, pair, qsl], osb[0][0:64, :], bc[0:64, :])
                nc.vector.tensor_mul(
                    ohT_sb[64:128, pair, qsl], osb[1][64:128, :], bc[64:128, :])

            # ---- main schedule ----
            # prologue: only the two qk chains attn(0,0) needs, drained;
            # the rest of prep(0) goes through the filler queue.
            g20 = Gen(emit_qk(2, 0))
            g00 = Gen(emit_qk(0, 0))
            drain(g20, g00)
            vs = [Gen(emit_v(st)) for st in range(4)]
            p1 = [Gen(emit_qk(3, 0)), Gen(emit_qk(1, 0))]
            for g in vs + p1:
                filler.append(g)

            y32 = wpool.tile([128, 8, 512], F32, tag="y32")
            for qc in range(NQC):
                nxt = queue_prep(qc + 1) if qc + 1 < NQC else None
                emit_attn(0, qc, vs)
                if qc == NQC - 1:
                    filler.append(Gen(emit_outproj_A(qc, y32)))
                drain(*p1)
                emit_attn(1, qc, vs)
                if qc == NQC - 1:
                    flush()
                    for _ in emit_outproj_B(qc, y32):
                        pass
                else:
                    filler.append(Gen(emit_outproj(qc)))
                if nxt is not None:
                    p0, vs, p1 = nxt
                    drain(*p0)  # the two chains attn(0, qc+1) starts on
            flush()

    nc.compile()
    return nc


def make_in_maps(query, W_in, W_out, sin_q, cos_q, attn_mask):
    bf = ml_dtypes.bfloat16
    cosT = np.asarray(cos_q, np.float32)[0, 0].T  # [64, S]
    sinT = np.asarray(sin_q, np.float32)[0, 0].T
    cosT_p = cosT[ROPE_PERM]
    sinT_p = sinT[ROPE_PERM] * ROPE_SGN[:, None]
    cos2 = np.concatenate([cosT_p, cosT_p], 0).astype(bf)    # [128, S]
    sin2 = np.concatenate([sinT_p, sinT_p], 0).astype(bf)
    W_in = np.asarray(W_in, np.float32)
    W_out = np.asarray(W_out, np.float32)
    query = np.asarray(query, np.float32)
    attn_mask = np.asarray(attn_mask)

    in_maps = []
    for c in range(NCORES):
        b, g = c // 4, c % 4
        heads = range(4 * g, 4 * g + 4)
        qrows = np.concatenate([W_in[h * 64:(h + 1) * 64][ROPE_PERM] for h in heads])
        krows = np.concatenate([W_in[TD + h * 64:TD + (h + 1) * 64][ROPE_PERM] for h in heads])
        vrows = np.concatenate([W_in[2 * TD + h * 64:2 * TD + (h + 1) * 64] for h in heads])
        tcols = np.concatenate([np.arange(h * 64, (h + 1) * 64) for h in heads])
        in_maps.append({
            "qT": np.ascontiguousarray(query[b].T).astype(bf),
            "wqkT": np.ascontiguousarray(np.concatenate([qrows, krows], 0).T).astype(bf),
            "wvT": np.ascontiguousarray(vrows.T).astype(bf),
            "cosT": cos2,
            "sinT": sin2,
            "maskv": np.ascontiguousarray(
                attn_mask[b].astype(np.float32).reshape(NKB, 128).T),
            "woutT": np.ascontiguousarray(W_out[:, tcols].T).astype(bf),
        })
    return in_maps


def _ensure_ntff_hook():
    """The image's antenv lacks axon_hooks; supply it so trace=True works."""
    try:
        from antenv.axon_hooks import get_axon_ntff_profile_hook  # noqa: F401
        return
    except ImportError:
        pass
    import types

    if "/root/.axon_site" not in sys.path:
        sys.path.insert(0, "/root/.axon_site")
    from trn_agent_boot.trn_boot import _ntff_profile_via_ctypes

    hook = _ntff_profile_via_ctypes("/opt/axon/libaxon_pjrt.so")
    mod = types.ModuleType("antenv.axon_hooks")
    mod._hook = hook
    mod.get_axon_ntff_profile_hook = lambda: mod._hook
    mod.set_axon_ntff_profile_hook = lambda h: setattr(mod, "_hook", h)
    sys.modules["antenv.axon_hooks"] = mod
    import antenv

    antenv.axon_hooks = mod


def kernel(query, W_in, W_out, sin_q, cos_q, attn_mask):
    mask = np.asarray(attn_mask)
    nvb = 1
    for b in range(B):
        idx = np.nonzero(mask[b])[0]
        last = int(idx[-1]) if idx.size else 0
        nvb = max(nvb, last // 128 + 1)
    key = ("nc", nvb)
    if key not in _CACHED:
        _CACHED[key] = build_program(nvb)
    nc = _CACHED[key]
    in_maps = make_in_maps(query, W_in, W_out, sin_q, cos_q, attn_mask)

    from concourse.bass_utils import run_bass_kernel_spmd

    trace = bool(os.environ.get("KERNEL_PROFILE"))
    if trace:
        try:
            _ensure_ntff_hook()
        except Exception as e:  # profiling is best-effort
            print(f"ntff hook unavailable: {e}")
            trace = False
    try:
        res = run_bass_kernel_spmd(nc, in_maps, list(range(NCORES)), trace=trace)
    except Exception:
        if not trace:
            raise
        res = run_bass_kernel_spmd(nc, in_maps, list(range(NCORES)), trace=False)
    _CACHED["last_result"] = res

    y = np.zeros((B, S, DM), np.float32)
    for c in range(NCORES):
        y[c // 4] += res.results[c]["yT"].astype(np.float32).T
    return y


# revision 23
# speedup vs baseline: 1.2954x; 1.0211x over previous
"""Trainium2 Bass kernel for nn_MultiHeadedAttention_71425306132929.

Fused QKV projection + RoPE + causal/padding-masked SDPA + output projection.

Sharding: 8 cores = 2 batches x 4 head-groups (4 heads each).  Each core
computes, for its (batch, head-group):
    qkT = (Wq|Wk) @ query[b].T      (transposed layout: head-dim on partitions)
    RoPE on qT/kT via in-quadrant partition shuffle (head dims permuted
    host-side so RoPE partners are 16 partitions apart)
    scoresT[k,q] = kT.T-dot-qT per head (2 heads packed via PE row tiling)
    PT = exp(scoresT * 1/8)  (no max-subtraction needed: logits are O(1))
    causal masking: block-skip + trimmed exp/matmul APs + affine_select on
    the diagonal 128-blocks
    padding mask: folded into v (zeroed rows) + an extra all-mask column that
    makes the attention-denominator fall out of the same matmul
    ohT = (v|m).T @ PT accumulated over key blocks -> unnormalized out + denom
    normalize via reciprocal on a DMA-partition-packed view + broadcast loads
    yT_partial = WoutT.T @ ohT  (row-parallel out-projection, bf16 store)
Host sums the 4 partial yT per batch.

Scheduling: the attention stream is ACT(exp)-paced; emission order software-
pipelines scores one block ahead of PV and interleaves the next q-chunk's
projection matmuls + the previous chunk's out-projection as PE filler fed
from a generator queue.  At round boundaries only the two qk chains the next
round's first scores need are drained; the rest keeps filling.
"""

import os
import sys
from collections import deque

import numpy as np

sys.path.insert(0, "/opt/trn_rl_repo")

import concourse.bass as bass  # noqa: E402
import concourse.bacc as bacc  # noqa: E402
import concourse.tile as tile  # noqa: E402
from concourse import mybir  # noqa: E402

import ml_dtypes  # noqa: E402

BF16 = mybir.dt.bfloat16
F32 = mybir.dt.float32

B, S, DM, TD, H, HD = 2, 2048, 1024, 1024, 16, 64
NCORES = 8
NH = 4          # heads per core
NKB = S // 128  # 16 key blocks
NQC = S // 512  # 4 query chunks
KC = DM // 128  # 8 contraction chunks

# RoPE partner permutation: place original dim d so that partner(p) = p ^ 16
# (within a 32-partition quadrant, reachable by DVE stream_shuffle).
ROPE_PERM = []
for _p in range(64):
    q32, r32 = _p // 32, _p % 32
    ROPE_PERM.append(q32 * 16 + r32 if r32 < 16 else 32 + q32 * 16 + (r32 - 16))
ROPE_SGN = np.array([-1.0 if (p % 32) < 16 else 1.0 for p in range(64)], np.float32)
SHUF_MASK = [i ^ 16 for i in range(32)]

_CACHED = {}


class Gen:
    """Steppable wrapper over an emission generator."""

    def __init__(self, g):
        self.g = g
        self.done = False

    def step(self):
        if self.done:
            return False
        try:
            next(self.g)
            return True
        except StopIteration:
            self.done = True
            return False


def build_program(nvb=NKB):
    nc = bacc.Bacc(None, target_bir_lowering=False)
    qT_d = nc.declare_dram_parameter("qT", [DM, S], BF16, isOutput=False)
    wqk_d = nc.declare_dram_parameter("wqkT", [DM, 512], BF16, isOutput=False)
    wv_d = nc.declare_dram_parameter("wvT", [DM, 256], BF16, isOutput=False)
    cos_d = nc.declare_dram_parameter("cosT", [128, S], BF16, isOutput=False)
    sin_d = nc.declare_dram_parameter("sinT", [128, S], BF16, isOutput=False)
    mkv_d = nc.declare_dram_parameter("maskv", [128, NKB], F32, isOutput=False)
    wo_d = nc.declare_dram_parameter("woutT", [256, DM], BF16, isOutput=False)
    yT_d = nc.declare_dram_parameter("yT", [DM, S], BF16, isOutput=True)
    dscr2 = nc.dram_tensor("rcp_scratch", [16, 512], F32)

    with tile.TileContext(nc) as tc:
        with (
            tc.tile_pool(name="const", bufs=1) as cpool,
            tc.tile_pool(name="work", bufs=1) as wpool,
            tc.tile_pool(name="rope", bufs=4) as rpool,
            tc.tile_pool(name="pt", bufs=8) as ptpool,
            tc.tile_pool(name="nrm", bufs=4) as npool,
            tc.tile_pool(name="yout", bufs=2) as ypool,
            tc.tile_pool(name="psA", bufs=2, space="PSUM") as psA,
            tc.tile_pool(name="psP", bufs=2, space="PSUM") as psP,
            tc.tile_pool(name="psO", bufs=2, space="PSUM") as psO,
        ):
            qT_sb = cpool.tile([128, KC, S], BF16, tag="qT")
            wqk_sb = cpool.tile([128, KC, 512], BF16, tag="wqk")
            wv_sb = cpool.tile([128, KC, 256], BF16, tag="wv")
            cos_sb = cpool.tile([128, S], BF16, tag="cos")
            sin_sb = cpool.tile([128, S], BF16, tag="sin")
            mkv_sb = cpool.tile([128, NKB], F32, tag="mkv")
            wo_sb = cpool.tile([128, 2, DM], BF16, tag="wo")

            qk_sb = wpool.tile([128, 4, S], BF16, tag="qk")
            vaug_sb = wpool.tile([128, NKB, 4, 128], BF16, tag="vaug")
            ohT_sb = wpool.tile([128, 2, S], BF16, tag="ohT")

            wqk_r = wqk_d.rearrange("(c p) s -> p c s", p=128)
            qT_r = qT_d.rearrange("(c p) s -> p c s", p=128)
            wv_r = wv_d.rearrange("(c p) s -> p c s", p=128)
            wo_r = wo_d.rearrange("(c p) s -> p c s", p=128)
            yT_r = yT_d.rearrange("(c p) s -> p c s", p=128)

            # Input DMA, merged into few descriptors, ordered so q-chunk
            # 0's projections can start ASAP.
            nc.sync.dma_start(mkv_sb[:], mkv_d[:])
            qsl0 = slice(0, 512)
            # first q-chunk's weights/activations land per-kc so the first
            # projection chains can chase the arriving chunks
            for kc in range(KC):
                nc.sync.dma_start(wqk_sb[:, kc, :], wqk_r[:, kc, :])
                nc.sync.dma_start(qT_sb[:, kc, qsl0], qT_r[:, kc, qsl0])
            nc.sync.dma_start(wv_sb[:], wv_r[:])
            nc.sync.dma_start(cos_sb[:], cos_d[:])
            nc.sync.dma_start(sin_sb[:], sin_d[:])
            for qn in range(1, NQC):
                qsl = slice(qn * 512, qn * 512 + 512)
                nc.sync.dma_start(qT_sb[:, :, qsl], qT_r[:, :, qsl])
            nc.sync.dma_start(wo_sb[:], wo_r[:])

            # HAM warm-up: dependency-free matmuls keep the PE activity
            # monitor busy from t~=6us so the real prologue runs at 2.4GHz.
            warm_sb = cpool.tile([128, 128], BF16, tag="warm")
            nc.gpsimd.memset(warm_sb[:], 0.0)
            warm_ps = psP.tile([128, 128], F32, tag="psP", name="warm_ps")
            for _w in range(96):
                nc.tensor.matmul(warm_ps[:], lhsT=warm_sb[:], rhs=warm_sb[:],
                                 start=True, stop=True, skip_group_check=True)

            nc.gpsimd.memset(vaug_sb[:], 0.0)
            # mask columns of v_aug: even slots col 64, odd slots col 32
            # (den must land on a legal engine start partition: 0/32/64/96)
            mkv_col = mkv_sb.rearrange("p (k o) -> p k o", o=1)
            nc.gpsimd.tensor_copy(vaug_sb[:, :, 0, 64:65], mkv_col)
            nc.gpsimd.tensor_copy(vaug_sb[:, :, 2, 64:65], mkv_col)
            nc.gpsimd.tensor_copy(vaug_sb[:, :, 1, 32:33], mkv_col)
            nc.gpsimd.tensor_copy(vaug_sb[:, :, 3, 32:33], mkv_col)

            def emit_qk(mt, qn):
                """project + rope one [128, 512] chunk of q or k (pair of heads).

                Generator: yields after each matmul so the caller can
                interleave; RoPE tail (DVE shuffle/mul + GPS mul/add) on close.
                """
                qsl = slice(qn * 512, qn * 512 + 512)
                ps = psP.tile([128, 512], F32, tag="psP")
                for kc in range(KC):
                    nc.tensor.matmul(
                        ps[:],
                        lhsT=wqk_sb[:, kc, mt * 128:(mt + 1) * 128],
                        rhs=qT_sb[:, kc, qsl],
                        start=(kc == 0),
                        stop=(kc == KC - 1),
                    )
                    if kc == 3:
                        yield
                qkp = rpool.tile([128, 512], BF16, tag="qkp")
                nc.vector.tensor_copy(qkp[:], ps[:])
                shuf = rpool.tile([128, 512], BF16, tag="shuf")
                nc.vector.stream_shuffle(shuf[:], qkp[:], mask=SHUF_MASK)
                t1 = rpool.tile([128, 512], BF16, tag="t1")
                nc.vector.tensor_mul(t1[:], qkp[:], cos_sb[:, qsl])
                t2 = rpool.tile([128, 512], BF16, tag="t2")
                nc.vector.tensor_mul(t2[:], shuf[:], sin_sb[:, qsl])
                nc.vector.tensor_add(qk_sb[:, mt, qsl], t1[:], t2[:])
                yield

            def emit_v(st):
                """project + mask one [128 keys, 4*64] v block into v_aug."""
                ps = psP.tile([128, 512], F32, tag="psP")
                psv = ps[:, 0:256]
                for kc in range(KC):
                    nc.tensor.matmul(
                        psv,
                        lhsT=qT_sb[:, kc, st * 128:(st + 1) * 128],
                        rhs=wv_sb[:, kc, :],
                        start=(kc == 0),
                        stop=(kc == KC - 1),
                    )
                    if kc == 3:
                        yield
                psv_h = psv.rearrange("p (h d) -> p h d", h=4)
                msk = mkv_sb[:, st:st + 1]
                # even local heads (slots 0,2) -> cols 0:64 ; odd -> cols 64:128
                nc.vector.tensor_scalar_mul(
                    vaug_sb[:, st, 0:4:2, 0:64], psv_h[:, 0:4:2, :], msk)
                nc.vector.tensor_scalar_mul(
                    vaug_sb[:, st, 1:4:2, 64:128], psv_h[:, 1:4:2, :], msk)
                yield

            def emit_outproj(qn):
                """row-parallel out-projection of one q chunk (bf16 store)."""
                qsl = slice(qn * 512, qn * 512 + 512)
                y = ypool.tile([128, 8, 512], BF16, tag="y")
                for mt in range(8):
                    ps = psP.tile([128, 512], F32, tag="psP")
                    for kc2 in range(2):
                        nc.tensor.matmul(
                            ps[:],
                            lhsT=wo_sb[:, kc2, mt * 128:(mt + 1) * 128],
                            rhs=ohT_sb[:, kc2, qsl],
                            start=(kc2 == 0),
                            stop=(kc2 == 1),
                        )
                    nc.any.tensor_copy(y[:, mt, :], ps[:])
                    if mt % 2 == 1:
                        nc.sync.dma_start(yT_r[:, mt - 1:mt + 1, qsl],
                                          y[:, mt - 1:mt + 1, :])
                    yield

            def emit_outproj_A(qn, y32):
                """pair-0 half of the last out-projection: runs during the
                final pair's attention, result parked in f32 SBUF."""
                qsl = slice(qn * 512, qn * 512 + 512)
                for mt in range(8):
                    ps = psP.tile([128, 512], F32, tag="psP")
                    nc.tensor.matmul(
                        ps[:],
                        lhsT=wo_sb[:, 0, mt * 128:(mt + 1) * 128],
                        rhs=ohT_sb[:, 0, qsl],
                        start=True, stop=True,
                    )
                    nc.any.tensor_copy(y32[:, mt, :], ps[:])
                    yield

            def emit_outproj_B(qn, y32):
                """pair-1 half + combine + store (the only post-norm work)."""
                qsl = slice(qn * 512, qn * 512 + 512)
                y = ypool.tile([128, 8, 512], BF16, tag="y")
                for mt in range(8):
                    ps = psP.tile([128, 512], F32, tag="psP")
                    nc.tensor.matmul(
                        ps[:],
                        lhsT=wo_sb[:, 1, mt * 128:(mt + 1) * 128],
                        rhs=ohT_sb[:, 1, qsl],
                        start=True, stop=True,
                    )
                    nc.vector.tensor_add(y[:, mt, :], y32[:, mt, :], ps[:])
                    if mt % 2 == 1:
                        nc.sync.dma_start(yT_r[:, mt - 1:mt + 1, qsl],
                                          y[:, mt - 1:mt + 1, :])
                    yield

            # ---- filler queue: projection / out-projection emission units
            # interleaved into the ACT-paced attention stream ----
            filler = deque()

            def pump(n):
                done = 0
                while done < n and filler:
                    if filler[0].step():
                        done += 1
                    else:
                        filler.popleft()

            def drain(*gens):
                for g in gens:
                    while g.step():
                        pass

            def flush():
                while filler:
                    if not filler[0].step():
                        filler.popleft()

            def queue_prep(qc):
                """queue next q-chunk's projections; returns handles:
                (pair0 qk chains, v chains, pair1 qk chains).  k/v chains
                past the padding-valid key range are never needed."""
                p0 = [Gen(emit_qk(0, qc))]
                p1 = [Gen(emit_qk(1, qc))]
                if qc * 512 < nvb * 128:
                    p0.insert(0, Gen(emit_qk(2, qc)))
                    p1.insert(0, Gen(emit_qk(3, qc)))
                vs = [Gen(emit_v(st))
                      for st in range(4 * qc, 4 * qc + 4) if st < nvb]
                for g in p0 + vs + p1:
                    filler.append(g)
                return p0, vs, p1

            def emit_attn(pair, qc, v_gens=None):
                """ACT-paced attention for one head-pair and q chunk.

                Software pipeline: scores one block ahead of PV; exp and
                matmul APs trimmed to the causally-valid q columns.
                v_gens: this chunk's diagonal v-projection chains — drained
                just before the first diagonal block's PV can need them.
                """
                nkb = min(4 * qc + 4, nvb)
                qmt, kmt = pair, 2 + pair
                oT = [psO.tile([128, 512], F32, tag="psO", name=f"oT{_h}")
                      for _h in range(2)]
                pts = [None] * nkb   # (pt tile, co) per block
                for kb in range(nkb):
                    if kb == max(4 * qc - 1, 0) and v_gens:
                        drain(*v_gens)
                    ksl = slice(kb * 128, kb * 128 + 128)
                    joff = kb - 4 * qc
                    co = max(joff, 0) * 128  # first causally-valid q col
                    st_ps = psA.tile([128, 1024], F32, tag="psA", name="stps")
                    for h in range(2):
                        pr = slice(64 * h, 64 * h + 64)
                        nc.tensor.matmul(
                            st_ps[:, h * 512 + co:(h + 1) * 512],
                            lhsT=qk_sb[pr, kmt, ksl],
                            rhs=qk_sb[pr, qmt, qc * 512 + co:qc * 512 + 512],
                            start=True,
                            stop=True,
                            skip_group_check=True,
                        )
                    pt = ptpool.tile([128, 1024], BF16, tag="pt", name="pt")
                    pts[kb] = (pt, co)
                    if co == 0:
                        nc.scalar.activation(
                            pt[:], st_ps[:],
                            mybir.ActivationFunctionType.Exp,
                            scale=0.125,
                        )
                    else:
                        # strided 2-range AP: only the valid q cols per head
                        pt_v = pt.rearrange("p (h q) -> p h q", h=2)
                        st_v = st_ps.rearrange("p (h q) -> p h q", h=2)
                        nc.scalar.activation(
                            pt_v[:, :, co:512], st_v[:, :, co:512],
                            mybir.ActivationFunctionType.Exp,
                            scale=0.125,
                        )
                    if joff >= 0:
                        for h in range(2):
                            nc.gpsimd.affine_select(
                                pt[:, h * 512 + co:h * 512 + co + 128],
                                pt[:, h * 512 + co:h * 512 + co + 128],
                                pattern=[[1, 128]],
                                compare_op=mybir.AluOpType.is_ge,
                                fill=0.0,
                                base=0,
                                channel_multiplier=-1,
                            )
                    if kb >= 1:
                        emit_pv(pair, kb - 1, pts[kb - 1], oT, nkb)
                    pump(1)
                emit_pv(pair, nkb - 1, pts[nkb - 1], oT, nkb)
                emit_norm(pair, qc, oT)

            def emit_pv(pair, kb, pt_co, oT, nkb):
                pt, co = pt_co
                for h in range(2):
                    nc.tensor.matmul(
                        oT[h][:, co:512],
                        lhsT=vaug_sb[:, kb, 2 * pair + h, :],
                        rhs=pt[:, h * 512 + co:(h + 1) * 512],
                        start=(kb == 0),
                        stop=(kb == nkb - 1),
                        skip_group_check=True,
                    )

            def emit_norm_fast(pair, qc, oT):
                """final-norm variant: SBUF pack -> reciprocal -> unpack ->
                GPSIMD partition_broadcast.  No DRAM bounce, single-packet
                DMAs — this chain is exposed at the kernel tail."""
                qsl = slice(qc * 512, qc * 512 + 512)
                osb = []
                for h in range(2):
                    o = npool.tile([128, 512], F32, tag="osb", name=f"osb{h}")
                    nc.vector.tensor_copy(o[:], oT[h][:])
                    osb.append(o)
                rcp = npool.tile([128, 8], F32, tag="rcp")
                nc.sync.dma_start(rcp[0:64, :], osb[0][64:65, :],
                                  single_packet=True)
                nc.sync.dma_start(rcp[64:128, :], osb[1][32:33, :],
                                  single_packet=True)
                rcp2 = npool.tile([128, 8], F32, tag="rcp2")
                nc.vector.reciprocal(rcp2[:], rcp[:])
                rowA = npool.tile([128, 512], F32, tag="rowA")
                rowB = npool.tile([128, 512], F32, tag="rowB")
                nc.sync.dma_start(rowA[0:1, :], rcp2[0:64, :],
                                  single_packet=True)
                nc.sync.dma_start(rowB[0:1, :], rcp2[64:128, :],
                                  single_packet=True)
                bc = npool.tile([128, 512], F32, tag="bc")
                nc.gpsimd.partition_broadcast(bc[0:64, :], rowA[0:1, :])
                nc.gpsimd.partition_broadcast(bc[64:128, :], rowB[0:1, :])
                nc.vector.tensor_mul(
                    ohT_sb[0:64, pair, qsl], osb[0][0:64, :], bc[0:64, :])
                nc.vector.tensor_mul(
                    ohT_sb[64:128, pair, qsl], osb[1][64:128, :], bc[64:128, :])

            def emit_norm(pair, qc, oT):
                """normalize: SBUF->SBUF DMA packs the two [1,512] den rows
                into [128,8] lanes for a cheap reciprocal, then a DRAM
                bounce provides the partition-broadcast.  No PE instruction
                in this chain: the in-order PE stream must never block on
                DMA latency."""
                qsl = slice(qc * 512, qc * 512 + 512)
                base = (pair * 4 + qc) * 2
                osb = []
                for h in range(2):
                    o = npool.tile([128, 512], F32, tag="osb", name=f"osb{h}")
                    nc.vector.tensor_copy(o[:], oT[h][:])  # frees the psum bank
                    osb.append(o)
                rcp = npool.tile([128, 8], F32, tag="rcp")
                nc.sync.dma_start(rcp[0:64, :], osb[0][64:65, :])
                nc.sync.dma_start(rcp[64:128, :], osb[1][32:33, :])
                rcp2 = npool.tile([128, 8], F32, tag="rcp2")
                nc.vector.reciprocal(rcp2[:], rcp[:])
                nc.sync.dma_start(
                    dscr2[base:base + 2, :].rearrange("a (p f) -> (a p) f", f=8), rcp2[:])
                bc = npool.tile([128, 512], F32, tag="bc")
                nc.gpsimd.dma_start(bc[0:64, :],
                                    dscr2[base:base + 1, :].to_broadcast((64, 512)))
                nc.gpsimd.dma_start(bc[64:128, :],
                                    dscr2[base + 1:base + 2, :].to_broadcast((64, 512)))
                nc.vector.tensor_mul(
                    ohT_sb[0:64, pair, qsl], osb[0][0:64, :], bc[0:64, :])
                nc.vector.tensor_mul(
                    ohT_sb[64:128, pair, qsl], osb[1][64:128, :], bc[64:128, :])

            # ---- main schedule ----
            # prologue: only the two qk chains attn(0,0) needs, drained;
            # the rest of prep(0) goes through the filler queue.
            g20 = Gen(emit_qk(2, 0))
            g00 = Gen(emit_qk(0, 0))
            drain(g20, g00)
            vs = [Gen(emit_v(st)) for st in range(4)]
            p1 = [Gen(emit_qk(3, 0)), Gen(emit_qk(1, 0))]
            for g in vs + p1:
                filler.append(g)

            y32 = wpool.tile([128, 8, 512], F32, tag="y32")
            for qc in range(NQC):
                nxt = queue_prep(qc + 1) if qc + 1 < NQC else None
                emit_attn(0, qc, vs)
                if qc == NQC - 1:
                    filler.append(Gen(emit_outproj_A(qc, y32)))
                drain(*p1)
                if nxt is not None:
                    # drain next round's pair-0 chains now: their RoPE tails
                    # land ahead of the norm/copy DVE backlog, so the next
                    # round's first scores aren't gated at the boundary
                    drain(*nxt[0])
                emit_attn(1, qc, vs)
                if qc == NQC - 1:
                    flush()
                    for _ in emit_outproj_B(qc, y32):
                        pass
                else:
                    filler.append(Gen(emit_outproj(qc)))
                if nxt is not None:
                    p0, vs, p1 = nxt
            flush()

    nc.compile()
    return nc


def make_in_maps(query, W_in, W_out, sin_q, cos_q, attn_mask):
    bf = ml_dtypes.bfloat16
    cosT = np.asarray(cos_q, np.float32)[0, 0].T  # [64, S]
    sinT = np.asarray(sin_q, np.float32)[0, 0].T
    cosT_p = cosT[ROPE_PERM]
    sinT_p = sinT[ROPE_PERM] * ROPE_SGN[:, None]
    cos2 = np.concatenate([cosT_p, cosT_p], 0).astype(bf)    # [128, S]
    sin2 = np.concatenate([sinT_p, sinT_p], 0).astype(bf)
    W_in = np.asarray(W_in, np.float32)
    W_out = np.asarray(W_out, np.float32)
    query = np.asarray(query, np.float32)
    attn_mask = np.asarray(attn_mask)

    in_maps = []
    for c in range(NCORES):
        b, g = c // 4, c % 4
        heads = range(4 * g, 4 * g + 4)
        qrows = np.concatenate([W_in[h * 64:(h + 1) * 64][ROPE_PERM] for h in heads])
        krows = np.concatenate([W_in[TD + h * 64:TD + (h + 1) * 64][ROPE_PERM] for h in heads])
        vrows = np.concatenate([W_in[2 * TD + h * 64:2 * TD + (h + 1) * 64] for h in heads])
        tcols = np.concatenate([np.arange(h * 64, (h + 1) * 64) for h in heads])
        in_maps.append({
            "qT": np.ascontiguousarray(query[b].T).astype(bf),
            "wqkT": np.ascontiguousarray(np.concatenate([qrows, krows], 0).T).astype(bf),
            "wvT": np.ascontiguousarray(vrows.T).astype(bf),
            "cosT": cos2,
            "sinT": sin2,
            "maskv": np.ascontiguousarray(
                attn_mask[b].astype(np.float32).reshape(NKB, 128).T),
            "woutT": np.ascontiguousarray(W_out[:, tcols].T).astype(bf),
        })
    return in_maps


def _ensure_ntff_hook():
    """The image's antenv lacks axon_hooks; supply it so trace=True works."""
    try:
        from antenv.axon_hooks import get_axon_ntff_profile_hook  # noqa: F401
        return
    except ImportError:
        pass
    import types

    if "/root/.axon_site" not in sys.path:
        sys.path.insert(0, "/root/.axon_site")
    from trn_agent_boot.trn_boot import _ntff_profile_via_ctypes

    hook = _ntff_profile_via_ctypes("/opt/axon/libaxon_pjrt.so")
    mod = types.ModuleType("antenv.axon_hooks")
    mod._hook = hook
    mod.get_axon_ntff_profile_hook = lambda: mod._hook
    mod.set_axon_ntff_profile_hook = lambda h: setattr(mod, "_hook", h)
    sys.modules["antenv.axon_hooks"] = mod
    import antenv

    antenv.axon_hooks = mod


def kernel(query, W_in, W_out, sin_q, cos_q, attn_mask):
    mask = np.asarray(attn_mask)
    nvb = 1
    for b in range(B):
        idx = np.nonzero(mask[b])[0]
        last = int(idx[-1]) if idx.size else 0
        nvb = max(nvb, last // 128 + 1)
    key = ("nc", nvb)
    if key not in _CACHED:
        _CACHED[key] = build_program(nvb)
    nc = _CACHED[key]
    in_maps = make_in_maps(query, W_in, W_out, sin_q, cos_q, attn_mask)

    from concourse.bass_utils import run_bass_kernel_spmd

    trace = bool(os.environ.get("KERNEL_PROFILE"))
    if trace:
        try:
            _ensure_ntff_hook()
        except Exception as e:  # profiling is best-effort
            print(f"ntff hook unavailable: {e}")
            trace = False
    try:
        res = run_bass_kernel_spmd(nc, in_maps, list(range(NCORES)), trace=trace)
    except Exception:
        if not trace:
            raise
        res = run_bass_kernel_spmd(nc, in_maps, list(range(NCORES)), trace=False)
    _CACHED["last_result"] = res

    y = np.zeros((B, S, DM), np.float32)
    for c in range(NCORES):
        y[c // 4] += res.results[c]["yT"].astype(np.float32).T
    return y
